# revision 5
# baseline (speedup 1.0000x reference)
"""GeneralAttention Trainium2 Bass kernel.

Computes, for each batch b (data-parallel, one batch per NeuronCore):
    key_t   = key @ W^T
    energy  = (query @ key_t^T) / sqrt(H)        [B, Lq, Lk]
    energy  = where(mask == 0, -1e10, energy)
    att     = softmax(energy, axis=-1)
    context = att @ value                        [B, Lq, H]
returns (context, attention).

Math used on-chip (identical up to fp rounding):
    energy  = (query @ (W/sqrt(H))) @ key^T      (transform Q, not K)
    p       = exp(energy + (mask-1)*1e10)        (masked entries -> exp(-1e10) == 0)
    att     = exp(energy + bias - ln(sum(p)))    (no row-max needed: |energy| < ~10)
    context = (p @ value) * (1/sum(p))
The additive mask is injected directly into the PSUM energy accumulation with
one extra matmul whose stationary operand is the 128x128 identity and whose
moving operand is the bias tile, so no vector-engine pass over [128, Lk] is
needed before the exponent.
"""

import math
import sys
from contextlib import ExitStack

for _p in ("/opt/trn_rl_repo",):
    if _p not in sys.path:
        sys.path.insert(0, _p)

import numpy as np

import concourse.bass as bass
import concourse.mybir as mybir
import concourse.tile as tile
from concourse.bass_utils import run_bass_kernel_spmd
from concourse.masks import make_identity

P = 128
NEG_SCALE = 1.0e10
F32 = mybir.dt.float32
BF16 = mybir.dt.bfloat16
I32 = mybir.dt.int32
Copy = mybir.ActivationFunctionType.Copy
Exp = mybir.ActivationFunctionType.Exp
Ln = mybir.ActivationFunctionType.Ln


_DMA_INSTS = (
    mybir.InstDMACopy,
    mybir.InstDMA,
    mybir.InstDmaTransposeAnt,
    mybir.InstDMAGatherAnt,
    mybir.InstDMAScatterAddAnt,
)


def _split_wide_sync_waits(nc, cap=1):
    """walrus in this container rejects >1 sync-wait command per instruction.
    Move excess waits onto preceding single-wait nops on the same engine —
    semantically identical (the sequencer stalls either way), just encoded
    across several instructions.  Excess sem updates on non-DMA instructions
    move to a following nop (same engine, in-order completion).  DMA updates
    are never moved (they fire at transfer completion, a nop would not)."""
    n_wait = n_upd = 0
    for f in nc.m.functions:
        for blk in f.blocks:
            new_insts = []
            for inst in blk.instructions:
                si = inst.sync_info
                if si is not None and si.on_wait and len(si.on_wait) > cap:
                    waits = list(si.on_wait)
                    for w in waits[:-cap]:
                        nop = mybir.InstNoOp(
                            name=f"waitsplit-{nc.next_id()}",
                            ins=[],
                            outs=[],
                            engine=inst.engine,
                            sync_info=mybir.SyncInfo(on_wait=[w], on_update=[]),
                        )
                        new_insts.append(nop)
                        n_wait += 1
                    si.on_wait = waits[-cap:]
                new_insts.append(inst)
                if (
                    si is not None
                    and si.on_update
                    and len(si.on_update) > cap
                    and not isinstance(inst, _DMA_INSTS)
                ):
                    upds = list(si.on_update)
                    si.on_update = upds[:cap]
                    for u in upds[cap:]:
                        nop = mybir.InstNoOp(
                            name=f"updsplit-{nc.next_id()}",
                            ins=[],
                            outs=[],
                            engine=inst.engine,
                            sync_info=mybir.SyncInfo(on_wait=[], on_update=[u]),
                        )
                        new_insts.append(nop)
                        n_upd += 1
            blk.instructions = new_insts
    return n_wait, n_upd


def build_attention_nc(Lq, Lk, H, mask_bias_on_act=True):
    assert Lq % 512 == 0 and Lk % 512 == 0 and H % 512 == 0
    HB, LqB, LkB = H // P, Lq // P, Lk // P
    KC, QC, HC2 = Lk // 512, Lq // 512, H // 512
    scale = 1.0 / math.sqrt(H)

    nc = bass.Bass(trn_type="TRN2")
    q_d = nc.dram_tensor("query", [Lq, H], F32, kind="ExternalInput")
    k_d = nc.dram_tensor("key", [Lk, H], F32, kind="ExternalInput")
    v_d = nc.dram_tensor("value", [Lk, H], F32, kind="ExternalInput")
    m_d = nc.dram_tensor("mask", [Lq, Lk], I32, kind="ExternalInput")
    w_d = nc.dram_tensor("W", [H, H], F32, kind="ExternalInput")
    ctx_d = nc.dram_tensor("context", [Lq, H], F32, kind="ExternalOutput")
    att_d = nc.dram_tensor("attention", [Lq, Lk], F32, kind="ExternalOutput")

    with tile.TileContext(nc) as tc, ExitStack() as ctx:
        persist = ctx.enter_context(tc.tile_pool(name="persist", bufs=1))
        # W natural layout (o-partition, o-chunk, h), pre-scaled by 1/sqrt(H)
        w_sb = persist.tile([P, HB, H], BF16)
        # value natural layout (k-partition, k-block, h)
        v_sb = persist.tile([P, LkB, H], BF16)
        # key^T (h-partition, h-chunk, k)
        kt_sb = persist.tile([P, HB, Lk], BF16)
        # (Q @ W/sqrt(H))^T (h-partition, h-chunk, q)
        qwt_sb = persist.tile([P, HB, Lq], BF16)
        id32 = persist.tile([P, P], F32)
        id16 = persist.tile([P, P], BF16)
        make_identity(nc, id32)
        make_identity(nc, id16)

        # ---------------- prep: W, V, K^T, q^T -> QW^T ----------------
        with (
            tc.tile_pool(name="prep_loads", bufs=2) as loads,
            tc.tile_pool(name="prep_misc", bufs=2) as prep_misc,
            tc.tile_pool(name="prep_ps", bufs=2, space="PSUM") as prep_ps,
            tc.tile_pool(name="qw_ps", bufs=2, space="PSUM") as qw_ps,
        ):
            for oc in range(HB):
                wf = prep_misc.tile([P, H], F32, tag="wf")
                nc.sync.dma_start(out=wf, in_=w_d[oc * P : (oc + 1) * P, :])
                nc.scalar.activation(w_sb[:, oc, :], wf, Copy, scale=scale)
            for kb in range(LkB):
                vf = prep_misc.tile([P, H], F32, tag="vf")
                nc.sync.dma_start(out=vf, in_=v_d[kb * P : (kb + 1) * P, :])
                nc.vector.tensor_copy(v_sb[:, kb, :], vf)

            for g in range(LkB // 4):
                kf = loads.tile([P, 4, H], F32, tag="ldf")
                nc.sync.dma_start(
                    out=kf,
                    in_=k_d[g * 512 : (g + 1) * 512, :].rearrange(
                        "(j p) h -> p j h", p=P
                    ),
                )
                for hc in range(HB):
                    ps = prep_ps.tile([P, 512], F32, tag="tps")
                    for j in range(4):
                        nc.tensor.transpose(
                            ps[:, j * P : (j + 1) * P],
                            kf[:, j, hc * P : (hc + 1) * P],
                            id32,
                        )
                    nc.vector.tensor_copy(kt_sb[:, hc, g * 512 : (g + 1) * 512], ps)

            for g in range(QC):
                qf = loads.tile([P, 4, H], F32, tag="ldf")
                nc.sync.dma_start(
                    out=qf,
                    in_=q_d[g * 512 : (g + 1) * 512, :].rearrange(
                        "(j p) h -> p j h", p=P
                    ),
                )
                qt = loads.tile([P, HB, 512], BF16, tag="qt")
                for oc in range(HB):
                    ps = prep_ps.tile([P, 512], F32, tag="tps")
                    for j in range(4):
                        nc.tensor.transpose(
                            ps[:, j * P : (j + 1) * P],
                            qf[:, j, oc * P : (oc + 1) * P],
                            id32,
                        )
                    nc.scalar.copy(qt[:, oc, :], ps)
                for hc in range(HB):
                    qw = qw_ps.tile([P, 512], F32, tag="qw")
                    for oc in range(HB):
                        nc.tensor.matmul(
                            qw,
                            w_sb[:, oc, hc * P : (hc + 1) * P],
                            qt[:, oc, :],
                            start=(oc == 0),
                            stop=(oc == HB - 1),
                        )
                    nc.scalar.copy(qwt_sb[:, hc, g * 512 : (g + 1) * 512], qw)

        # ---------------- main loop over q row-blocks ----------------
        with (
            tc.tile_pool(name="maskp", bufs=2) as maskp,
            tc.tile_pool(name="biasp", bufs=2) as biasp,
            tc.tile_pool(name="pp", bufs=2) as pp,
            tc.tile_pool(name="attp", bufs=2) as attp,
            tc.tile_pool(name="ptp", bufs=2) as ptp,
            tc.tile_pool(name="ctxp", bufs=2) as ctxp,
            tc.tile_pool(name="sums", bufs=3) as sums,
            tc.tile_pool(name="e_ps", bufs=1, space="PSUM") as e_ps,
            tc.tile_pool(name="pt_ps", bufs=2, space="PSUM") as pt_ps,
            tc.tile_pool(name="c_ps", bufs=2, space="PSUM") as c_ps,
        ):
            for qb in range(LqB):
                qsl = slice(qb * P, (qb + 1) * P)
                mk = maskp.tile([P, Lk], I32)
                nc.sync.dma_start(out=mk, in_=m_d[qsl, :])
                bias = biasp.tile([P, Lk], BF16)
                if mask_bias_on_act:
                    nc.scalar.activation(
                        bias, mk, Copy, bias=-NEG_SCALE, scale=NEG_SCALE
                    )
                else:
                    nc.vector.tensor_scalar(
                        bias,
                        mk,
                        NEG_SCALE,
                        -NEG_SCALE,
                        mybir.AluOpType.mult,
                        mybir.AluOpType.add,
                    )

                eng = e_ps.tile([P, Lk], F32)
                for kc in range(KC):
                    ksl = slice(kc * 512, (kc + 1) * 512)
                    for hc in range(HB):
                        nc.tensor.matmul(
                            eng[:, ksl],
                            qwt_sb[:, hc, qsl],
                            kt_sb[:, hc, ksl],
                            start=(hc == 0),
                            stop=False,
                        )
                    nc.tensor.matmul(
                        eng[:, ksl], id16, bias[:, ksl], start=False, stop=True
                    )

                p16 = pp.tile([P, Lk], BF16)
                rsum = sums.tile([P, 1], F32, tag="rsum")
                nc.scalar.activation(p16, eng, Exp, accum_out=rsum)
                lns = sums.tile([P, 1], F32, tag="lns")
                nc.scalar.activation(lns, rsum, Ln)
                negln = sums.tile([P, 1], F32, tag="negln")
                nc.vector.tensor_scalar_mul(negln, lns, -1.0)
                recip = sums.tile([P, 1], F32, tag="recip")
                nc.vector.reciprocal(recip, rsum)

                att = attp.tile([P, Lk], F32)
                nc.scalar.activation(att, eng, Exp, bias=negln)
                nc.scalar.dma_start(out=att_d[qsl, :], in_=att)

                pt = ptp.tile([P, LkB, P], BF16)
                for g4 in range(LkB // 4):
                    ps = pt_ps.tile([P, 512], BF16, tag="ptps")
                    for j in range(4):
                        kb = g4 * 4 + j
                        nc.tensor.transpose(
                            ps[:, j * P : (j + 1) * P],
                            p16[:, kb * P : (kb + 1) * P],
                            id16,
                        )
                    nc.vector.tensor_copy(pt[:, g4 * 4 : (g4 + 1) * 4, :], ps)

                ctx_sb = ctxp.tile([P, H], F32)
                for h2 in range(HC2):
                    hsl = slice(h2 * 512, (h2 + 1) * 512)
                    cps = c_ps.tile([P, 512], F32, tag="cps")
                    for kb in range(LkB):
                        nc.tensor.matmul(
                            cps,
                            pt[:, kb, :],
                            v_sb[:, kb, hsl],
                            start=(kb == 0),
                            stop=(kb == LkB - 1),
                        )
                    nc.vector.tensor_scalar_mul(ctx_sb[:, hsl], cps, recip)
                nc.scalar.dma_start(out=ctx_d[qsl, :], in_=ctx_sb)

    _split_wide_sync_waits(nc)
    return nc


_nc_cache = {}


def _get_nc(Lq, Lk, H):
    key = (Lq, Lk, H)
    if key not in _nc_cache:
        _nc_cache[key] = build_attention_nc(Lq, Lk, H)
    return _nc_cache[key]


def kernel(query, key, value, mask, W, trace=False):
    query = np.ascontiguousarray(np.asarray(query, dtype=np.float32))
    key = np.ascontiguousarray(np.asarray(key, dtype=np.float32))
    value = np.ascontiguousarray(np.asarray(value, dtype=np.float32))
    mask = np.ascontiguousarray(np.asarray(mask, dtype=np.int32))
    W = np.ascontiguousarray(np.asarray(W, dtype=np.float32))

    B, Lq, H = query.shape
    Lk = key.shape[1]
    assert B == 8, f"expected B=8, got {B}"

    nc = _get_nc(Lq, Lk, H)
    in_maps = [
        {
            "query": query[b],
            "key": key[b],
            "value": value[b],
            "mask": mask[b],
            "W": W,
        }
        for b in range(B)
    ]
    res = run_bass_kernel_spmd(
        nc, in_maps, core_ids=list(range(B)), trace=trace
    )
    context = np.stack([r["context"] for r in res.results])
    attention = np.stack([r["attention"] for r in res.results])
    if trace:
        kernel.last_exec_time_ns = res.exec_time_ns
        kernel.last_results = res
    return context, attention


# revision 9
# speedup vs baseline: 1.0266x; 1.0266x over previous
"""GeneralAttention Trainium2 Bass kernel.

Computes, for each batch b (data-parallel, one batch per NeuronCore):
    key_t   = key @ W^T
    energy  = (query @ key_t^T) / sqrt(H)        [B, Lq, Lk]
    energy  = where(mask == 0, -1e10, energy)
    att     = softmax(energy, axis=-1)
    context = att @ value                        [B, Lq, H]
returns (context, attention).

Math used on-chip (identical up to fp rounding):
    energy  = (query @ (W/sqrt(H))) @ key^T      (transform Q, not K)
    p       = exp(energy + (mask-1)*1e10)        (masked entries -> exp(-1e10) == 0)
    att     = exp(energy + bias - ln(sum(p)))    (no row-max needed: |energy| < ~10)
    context = (p @ value) * (1/sum(p))
The additive mask is injected directly into the PSUM energy accumulation with
one extra matmul whose stationary operand is the 128x128 identity and whose
moving operand is the bias tile, so no vector-engine pass over [128, Lk] is
needed before the exponent.
"""

import math
import sys
from contextlib import ExitStack

for _p in ("/opt/trn_rl_repo",):
    if _p not in sys.path:
        sys.path.insert(0, _p)

import numpy as np

import concourse.bass as bass
import concourse.mybir as mybir
import concourse.tile as tile
from concourse.bass_utils import run_bass_kernel_spmd
from concourse.masks import make_identity

P = 128
NEG_SCALE = 1.0e10
F32 = mybir.dt.float32
BF16 = mybir.dt.bfloat16
I32 = mybir.dt.int32
Copy = mybir.ActivationFunctionType.Copy
Exp = mybir.ActivationFunctionType.Exp
Ln = mybir.ActivationFunctionType.Ln


_DMA_INSTS = (
    mybir.InstDMACopy,
    mybir.InstDMA,
    mybir.InstDmaTransposeAnt,
    mybir.InstDMAGatherAnt,
    mybir.InstDMAScatterAddAnt,
)


def _split_wide_sync_waits(nc, cap=1):
    """walrus in this container rejects >1 sync-wait command per instruction.
    Move excess waits onto preceding single-wait nops on the same engine —
    semantically identical (the sequencer stalls either way), just encoded
    across several instructions.  Excess sem updates on non-DMA instructions
    move to a following nop (same engine, in-order completion).  DMA updates
    are never moved (they fire at transfer completion, a nop would not)."""
    n_wait = n_upd = 0
    for f in nc.m.functions:
        for blk in f.blocks:
            new_insts = []
            for inst in blk.instructions:
                si = inst.sync_info
                if si is not None and si.on_wait and len(si.on_wait) > cap:
                    waits = list(si.on_wait)
                    for w in waits[:-cap]:
                        nop = mybir.InstNoOp(
                            name=f"waitsplit-{nc.next_id()}",
                            ins=[],
                            outs=[],
                            engine=inst.engine,
                            sync_info=mybir.SyncInfo(on_wait=[w], on_update=[]),
                        )
                        new_insts.append(nop)
                        n_wait += 1
                    si.on_wait = waits[-cap:]
                new_insts.append(inst)
                if (
                    si is not None
                    and si.on_update
                    and len(si.on_update) > cap
                    and not isinstance(inst, _DMA_INSTS)
                ):
                    upds = list(si.on_update)
                    si.on_update = upds[:cap]
                    for u in upds[cap:]:
                        nop = mybir.InstNoOp(
                            name=f"updsplit-{nc.next_id()}",
                            ins=[],
                            outs=[],
                            engine=inst.engine,
                            sync_info=mybir.SyncInfo(on_wait=[], on_update=[u]),
                        )
                        new_insts.append(nop)
                        n_upd += 1
            blk.instructions = new_insts
    return n_wait, n_upd


def build_attention_nc(Lq, Lk, H, mask_bias_on_act=True):
    assert Lq % 512 == 0 and Lk % 512 == 0 and H % 512 == 0
    HB, LqB, LkB = H // P, Lq // P, Lk // P
    KC, QC, HC2 = Lk // 512, Lq // 512, H // 512
    scale = 1.0 / math.sqrt(H)

    nc = bass.Bass(trn_type="TRN2")
    q_d = nc.dram_tensor("query", [Lq, H], F32, kind="ExternalInput")
    k_d = nc.dram_tensor("key", [Lk, H], F32, kind="ExternalInput")
    v_d = nc.dram_tensor("value", [Lk, H], F32, kind="ExternalInput")
    m_d = nc.dram_tensor("mask", [Lq, Lk], I32, kind="ExternalInput")
    w_d = nc.dram_tensor("W", [H, H], F32, kind="ExternalInput")
    ctx_d = nc.dram_tensor("context", [Lq, H], F32, kind="ExternalOutput")
    att_d = nc.dram_tensor("attention", [Lq, Lk], F32, kind="ExternalOutput")

    with tile.TileContext(nc) as tc, ExitStack() as ctx:
        persist = ctx.enter_context(tc.tile_pool(name="persist", bufs=1))
        # W natural layout (o-partition, o-chunk, h), pre-scaled by 1/sqrt(H)
        w_sb = persist.tile([P, HB, H], BF16)
        # value natural layout (k-partition, k-block, h)
        v_sb = persist.tile([P, LkB, H], BF16)
        # key^T (h-partition, h-chunk, k)
        kt_sb = persist.tile([P, HB, Lk], BF16)
        # (Q @ W/sqrt(H))^T (h-partition, h-chunk, q)
        qwt_sb = persist.tile([P, HB, Lq], BF16)
        id32 = persist.tile([P, P], F32)
        id16 = persist.tile([P, P], BF16)
        make_identity(nc, id32)
        make_identity(nc, id16)

        # ---------------- prep: W, V, K^T, q^T -> QW^T ----------------
        with (
            tc.tile_pool(name="prep_loads", bufs=2) as loads,
            tc.tile_pool(name="prep_misc", bufs=2) as prep_misc,
            tc.tile_pool(name="prep_ps", bufs=2, space="PSUM") as prep_ps,
            tc.tile_pool(name="qw_ps", bufs=2, space="PSUM") as qw_ps,
        ):
            # DMA queues are FIFO per HWDGE engine, and emission order sets the
            # scheduler's tie-break priority — so load in critical-path order:
            # K (feeds K^T transposes, gates every energy matmul), then W and
            # Q (feed QW^T), then V (only needed for the first context matmul
            # ~150us in).  Mask rides the scalar-engine queue (see main loop).
            for g in range(LkB // 4):
                kf = loads.tile([P, 4, H], F32, tag="ldf")
                nc.sync.dma_start(
                    out=kf,
                    in_=k_d[g * 512 : (g + 1) * 512, :].rearrange(
                        "(j p) h -> p j h", p=P
                    ),
                )
                for hc in range(HB):
                    ps = prep_ps.tile([P, 512], F32, tag="tps")
                    for j in range(4):
                        nc.tensor.transpose(
                            ps[:, j * P : (j + 1) * P],
                            kf[:, j, hc * P : (hc + 1) * P],
                            id32,
                        )
                    nc.vector.tensor_copy(kt_sb[:, hc, g * 512 : (g + 1) * 512], ps)

            for oc in range(HB):
                wf = prep_misc.tile([P, H], F32, tag="wf")
                nc.sync.dma_start(out=wf, in_=w_d[oc * P : (oc + 1) * P, :])
                nc.scalar.activation(w_sb[:, oc, :], wf, Copy, scale=scale)

            for g in range(QC):
                qf = loads.tile([P, 4, H], F32, tag="ldf")
                nc.sync.dma_start(
                    out=qf,
                    in_=q_d[g * 512 : (g + 1) * 512, :].rearrange(
                        "(j p) h -> p j h", p=P
                    ),
                )
                qt = loads.tile([P, HB, 512], BF16, tag="qt")
                for oc in range(HB):
                    ps = prep_ps.tile([P, 512], F32, tag="tps")
                    for j in range(4):
                        nc.tensor.transpose(
                            ps[:, j * P : (j + 1) * P],
                            qf[:, j, oc * P : (oc + 1) * P],
                            id32,
                        )
                    nc.scalar.copy(qt[:, oc, :], ps)
                for hc in range(HB):
                    qw = qw_ps.tile([P, 512], F32, tag="qw")
                    for oc in range(HB):
                        nc.tensor.matmul(
                            qw,
                            w_sb[:, oc, hc * P : (hc + 1) * P],
                            qt[:, oc, :],
                            start=(oc == 0),
                            stop=(oc == HB - 1),
                        )
                    nc.scalar.copy(qwt_sb[:, hc, g * 512 : (g + 1) * 512], qw)

            for kb in range(LkB):
                vf = prep_misc.tile([P, H], F32, tag="vf")
                nc.sync.dma_start(out=vf, in_=v_d[kb * P : (kb + 1) * P, :])
                nc.vector.tensor_copy(v_sb[:, kb, :], vf)

        # ---------------- main loop over q row-blocks ----------------
        with (
            tc.tile_pool(name="maskp", bufs=2) as maskp,
            tc.tile_pool(name="biasp", bufs=2) as biasp,
            tc.tile_pool(name="pp", bufs=2) as pp,
            tc.tile_pool(name="attp", bufs=2) as attp,
            tc.tile_pool(name="ptp", bufs=2) as ptp,
            tc.tile_pool(name="ctxp", bufs=2) as ctxp,
            tc.tile_pool(name="sums", bufs=3) as sums,
            tc.tile_pool(name="e_ps", bufs=1, space="PSUM") as e_ps,
            tc.tile_pool(name="pt_ps", bufs=2, space="PSUM") as pt_ps,
            tc.tile_pool(name="c_ps", bufs=2, space="PSUM") as c_ps,
        ):
            for qb in range(LqB):
                qsl = slice(qb * P, (qb + 1) * P)
                mk = maskp.tile([P, Lk], I32)
                nc.scalar.dma_start(out=mk, in_=m_d[qsl, :])
                bias = biasp.tile([P, Lk], BF16)
                if mask_bias_on_act:
                    nc.scalar.activation(
                        bias, mk, Copy, bias=-NEG_SCALE, scale=NEG_SCALE
                    )
                else:
                    nc.vector.tensor_scalar(
                        bias,
                        mk,
                        NEG_SCALE,
                        -NEG_SCALE,
                        mybir.AluOpType.mult,
                        mybir.AluOpType.add,
                    )

                eng = e_ps.tile([P, Lk], F32)
                for kc in range(KC):
                    ksl = slice(kc * 512, (kc + 1) * 512)
                    for hc in range(HB):
                        nc.tensor.matmul(
                            eng[:, ksl],
                            qwt_sb[:, hc, qsl],
                            kt_sb[:, hc, ksl],
                            start=(hc == 0),
                            stop=False,
                        )
                    nc.tensor.matmul(
                        eng[:, ksl], id16, bias[:, ksl], start=False, stop=True
                    )

                p16 = pp.tile([P, Lk], BF16)
                rsum = sums.tile([P, 1], F32, tag="rsum")
                nc.scalar.activation(p16, eng, Exp, accum_out=rsum)
                lns = sums.tile([P, 1], F32, tag="lns")
                nc.scalar.activation(lns, rsum, Ln)
                negln = sums.tile([P, 1], F32, tag="negln")
                nc.vector.tensor_scalar_mul(negln, lns, -1.0)
                recip = sums.tile([P, 1], F32, tag="recip")
                nc.vector.reciprocal(recip, rsum)

                att = attp.tile([P, Lk], F32)
                nc.scalar.activation(att, eng, Exp, bias=negln)
                nc.scalar.dma_start(out=att_d[qsl, :], in_=att)

                pt = ptp.tile([P, LkB, P], BF16)
                for g4 in range(LkB // 4):
                    ps = pt_ps.tile([P, 512], BF16, tag="ptps")
                    for j in range(4):
                        kb = g4 * 4 + j
                        nc.tensor.transpose(
                            ps[:, j * P : (j + 1) * P],
                            p16[:, kb * P : (kb + 1) * P],
                            id16,
                        )
                    nc.vector.tensor_copy(pt[:, g4 * 4 : (g4 + 1) * 4, :], ps)

                ctx_sb = ctxp.tile([P, H], F32)
                for h2 in range(HC2):
                    hsl = slice(h2 * 512, (h2 + 1) * 512)
                    cps = c_ps.tile([P, 512], F32, tag="cps")
                    for kb in range(LkB):
                        nc.tensor.matmul(
                            cps,
                            pt[:, kb, :],
                            v_sb[:, kb, hsl],
                            start=(kb == 0),
                            stop=(kb == LkB - 1),
                        )
                    nc.vector.tensor_scalar_mul(ctx_sb[:, hsl], cps, recip)
                nc.scalar.dma_start(out=ctx_d[qsl, :], in_=ctx_sb)

    _split_wide_sync_waits(nc)
    return nc


_nc_cache = {}


def _get_nc(Lq, Lk, H):
    key = (Lq, Lk, H)
    if key not in _nc_cache:
        _nc_cache[key] = build_attention_nc(Lq, Lk, H)
    return _nc_cache[key]


def kernel(query, key, value, mask, W, trace=False):
    query = np.ascontiguousarray(np.asarray(query, dtype=np.float32))
    key = np.ascontiguousarray(np.asarray(key, dtype=np.float32))
    value = np.ascontiguousarray(np.asarray(value, dtype=np.float32))
    mask = np.ascontiguousarray(np.asarray(mask, dtype=np.int32))
    W = np.ascontiguousarray(np.asarray(W, dtype=np.float32))

    B, Lq, H = query.shape
    Lk = key.shape[1]
    assert B == 8, f"expected B=8, got {B}"

    nc = _get_nc(Lq, Lk, H)
    in_maps = [
        {
            "query": query[b],
            "key": key[b],
            "value": value[b],
            "mask": mask[b],
            "W": W,
        }
        for b in range(B)
    ]
    res = run_bass_kernel_spmd(
        nc, in_maps, core_ids=list(range(B)), trace=trace
    )
    context = np.stack([r["context"] for r in res.results])
    attention = np.stack([r["attention"] for r in res.results])
    if trace:
        kernel.last_exec_time_ns = res.exec_time_ns
        kernel.last_results = res
    return context, attention


# revision 10
# speedup vs baseline: 1.0732x; 1.0454x over previous
"""GeneralAttention Trainium2 Bass kernel.

Computes, for each batch b (data-parallel, one batch per NeuronCore):
    key_t   = key @ W^T
    energy  = (query @ key_t^T) / sqrt(H)        [B, Lq, Lk]
    energy  = where(mask == 0, -1e10, energy)
    att     = softmax(energy, axis=-1)
    context = att @ value                        [B, Lq, H]
returns (context, attention).

Math used on-chip (identical up to fp rounding):
    energy  = (query @ (W/sqrt(H))) @ key^T      (transform Q, not K)
    p       = exp(energy + (mask-1)*1e10)        (masked entries -> exp(-1e10) == 0)
    att     = exp(energy + bias - ln(sum(p)))    (no row-max needed: |energy| < ~10)
    context = (p @ value) * (1/sum(p))
The additive mask is injected directly into the PSUM energy accumulation with
one extra matmul whose stationary operand is the 128x128 identity and whose
moving operand is the bias tile, so no vector-engine pass over [128, Lk] is
needed before the exponent.
"""

import math
import sys
from contextlib import ExitStack

for _p in ("/opt/trn_rl_repo",):
    if _p not in sys.path:
        sys.path.insert(0, _p)

import numpy as np

import concourse.bass as bass
import concourse.mybir as mybir
import concourse.tile as tile
from concourse.bass_utils import run_bass_kernel_spmd
from concourse.masks import make_identity

P = 128
NEG_SCALE = 1.0e10
F32 = mybir.dt.float32
BF16 = mybir.dt.bfloat16
I32 = mybir.dt.int32
Copy = mybir.ActivationFunctionType.Copy
Exp = mybir.ActivationFunctionType.Exp
Ln = mybir.ActivationFunctionType.Ln


_DMA_INSTS = (
    mybir.InstDMACopy,
    mybir.InstDMA,
    mybir.InstDmaTransposeAnt,
    mybir.InstDMAGatherAnt,
    mybir.InstDMAScatterAddAnt,
)


def _split_wide_sync_waits(nc, cap=1):
    """walrus in this container rejects >1 sync-wait command per instruction.
    Move excess waits onto preceding single-wait nops on the same engine —
    semantically identical (the sequencer stalls either way), just encoded
    across several instructions.  Excess sem updates on non-DMA instructions
    move to a following nop (same engine, in-order completion).  DMA updates
    are never moved (they fire at transfer completion, a nop would not)."""
    n_wait = n_upd = 0
    for f in nc.m.functions:
        for blk in f.blocks:
            new_insts = []
            for inst in blk.instructions:
                si = inst.sync_info
                if si is not None and si.on_wait and len(si.on_wait) > cap:
                    waits = list(si.on_wait)
                    for w in waits[:-cap]:
                        nop = mybir.InstNoOp(
                            name=f"waitsplit-{nc.next_id()}",
                            ins=[],
                            outs=[],
                            engine=inst.engine,
                            sync_info=mybir.SyncInfo(on_wait=[w], on_update=[]),
                        )
                        new_insts.append(nop)
                        n_wait += 1
                    si.on_wait = waits[-cap:]
                new_insts.append(inst)
                if (
                    si is not None
                    and si.on_update
                    and len(si.on_update) > cap
                    and not isinstance(inst, _DMA_INSTS)
                ):
                    upds = list(si.on_update)
                    si.on_update = upds[:cap]
                    for u in upds[cap:]:
                        nop = mybir.InstNoOp(
                            name=f"updsplit-{nc.next_id()}",
                            ins=[],
                            outs=[],
                            engine=inst.engine,
                            sync_info=mybir.SyncInfo(on_wait=[], on_update=[u]),
                        )
                        new_insts.append(nop)
                        n_upd += 1
            blk.instructions = new_insts
    return n_wait, n_upd


def build_attention_nc(Lq, Lk, H, mask_bias_on_act=True):
    assert Lq % 512 == 0 and Lk % 512 == 0 and H % 512 == 0
    HB, LqB, LkB = H // P, Lq // P, Lk // P
    KC, QC, HC2 = Lk // 512, Lq // 512, H // 512
    scale = 1.0 / math.sqrt(H)

    nc = bass.Bass(trn_type="TRN2")
    q_d = nc.dram_tensor("query", [Lq, H], F32, kind="ExternalInput")
    k_d = nc.dram_tensor("key", [Lk, H], F32, kind="ExternalInput")
    v_d = nc.dram_tensor("value", [Lk, H], F32, kind="ExternalInput")
    m_d = nc.dram_tensor("mask", [Lq, Lk], I32, kind="ExternalInput")
    w_d = nc.dram_tensor("W", [H, H], F32, kind="ExternalInput")
    ctx_d = nc.dram_tensor("context", [Lq, H], F32, kind="ExternalOutput")
    att_d = nc.dram_tensor("attention", [Lq, Lk], F32, kind="ExternalOutput")

    with tile.TileContext(nc) as tc, ExitStack() as ctx:
        persist = ctx.enter_context(tc.tile_pool(name="persist", bufs=1))
        # W natural layout (o-partition, o-chunk, h), pre-scaled by 1/sqrt(H)
        w_sb = persist.tile([P, HB, H], BF16)
        # value natural layout (k-partition, k-block, h)
        v_sb = persist.tile([P, LkB, H], BF16)
        # key^T (h-partition, h-chunk, k)
        kt_sb = persist.tile([P, HB, Lk], BF16)
        # (Q @ W/sqrt(H))^T (h-partition, h-chunk, q)
        qwt_sb = persist.tile([P, HB, Lq], BF16)
        id32 = persist.tile([P, P], F32)
        id16 = persist.tile([P, P], BF16)
        make_identity(nc, id32)
        make_identity(nc, id16)

        # ---------------- prep: W, V, K^T, q^T -> QW^T ----------------
        with (
            tc.tile_pool(name="prep_loads", bufs=2) as loads,
            tc.tile_pool(name="prep_misc", bufs=2) as prep_misc,
            tc.tile_pool(name="prep_ps", bufs=2, space="PSUM") as prep_ps,
            tc.tile_pool(name="qw_ps", bufs=2, space="PSUM") as qw_ps,
        ):
            # DMA queues are FIFO per HWDGE engine, and emission order sets
            # the scheduler's tie-break priority — so load in critical-path
            # order: first K-group 0 and W and Q-group 0 (together they
            # unblock the first energy matmuls ~30us in), then the remaining
            # K and Q groups, then V (only needed for the first context
            # matmul much later).  Mask rides the scalar-engine queue.
            def k_group(g):
                kf = loads.tile([P, 4, H], F32, tag="ldf", name="kf")
                nc.sync.dma_start(
                    out=kf,
                    in_=k_d[g * 512 : (g + 1) * 512, :].rearrange(
                        "(j p) h -> p j h", p=P
                    ),
                )
                for hc in range(HB):
                    ps = prep_ps.tile([P, 512], F32, tag="tps", name="ps")
                    for j in range(4):
                        nc.tensor.transpose(
                            ps[:, j * P : (j + 1) * P],
                            kf[:, j, hc * P : (hc + 1) * P],
                            id32,
                        )
                    nc.vector.tensor_copy(kt_sb[:, hc, g * 512 : (g + 1) * 512], ps)

            def q_group(g):
                qf = loads.tile([P, 4, H], F32, tag="ldf", name="qf")
                nc.sync.dma_start(
                    out=qf,
                    in_=q_d[g * 512 : (g + 1) * 512, :].rearrange(
                        "(j p) h -> p j h", p=P
                    ),
                )
                qt = loads.tile([P, HB, 512], BF16, tag="qt", name="qt")
                for oc in range(HB):
                    ps = prep_ps.tile([P, 512], F32, tag="tps", name="ps")
                    for j in range(4):
                        nc.tensor.transpose(
                            ps[:, j * P : (j + 1) * P],
                            qf[:, j, oc * P : (oc + 1) * P],
                            id32,
                        )
                    nc.scalar.copy(qt[:, oc, :], ps)
                for hc in range(HB):
                    qw = qw_ps.tile([P, 512], F32, tag="qw", name="qw")
                    for oc in range(HB):
                        nc.tensor.matmul(
                            qw,
                            w_sb[:, oc, hc * P : (hc + 1) * P],
                            qt[:, oc, :],
                            start=(oc == 0),
                            stop=(oc == HB - 1),
                        )
                    nc.scalar.copy(qwt_sb[:, hc, g * 512 : (g + 1) * 512], qw)

            k_group(0)
            for oc in range(HB):
                wf = prep_misc.tile([P, H], F32, tag="wf")
                nc.sync.dma_start(out=wf, in_=w_d[oc * P : (oc + 1) * P, :])
                nc.scalar.activation(w_sb[:, oc, :], wf, Copy, scale=scale)
            q_group(0)
            for g in range(1, LkB // 4):
                k_group(g)
                q_group(g)
            for g in range(LkB // 4, QC):
                q_group(g)

            for kb in range(LkB):
                vf = prep_misc.tile([P, H], F32, tag="vf")
                nc.sync.dma_start(out=vf, in_=v_d[kb * P : (kb + 1) * P, :])
                nc.vector.tensor_copy(v_sb[:, kb, :], vf)

        # ---------------- main loop over q row-blocks ----------------
        with (
            tc.tile_pool(name="maskp", bufs=2) as maskp,
            tc.tile_pool(name="biasp", bufs=2) as biasp,
            tc.tile_pool(name="pp", bufs=2) as pp,
            tc.tile_pool(name="attp", bufs=2) as attp,
            tc.tile_pool(name="ptp", bufs=2) as ptp,
            tc.tile_pool(name="ctxp", bufs=2) as ctxp,
            tc.tile_pool(name="sums", bufs=3) as sums,
            tc.tile_pool(name="e_ps", bufs=1, space="PSUM") as e_ps,
            tc.tile_pool(name="pt_ps", bufs=2, space="PSUM") as pt_ps,
            tc.tile_pool(name="c_ps", bufs=2, space="PSUM") as c_ps,
        ):
            for qb in range(LqB):
                qsl = slice(qb * P, (qb + 1) * P)
                mk = maskp.tile([P, Lk], I32)
                nc.scalar.dma_start(out=mk, in_=m_d[qsl, :])
                bias = biasp.tile([P, Lk], BF16)
                if mask_bias_on_act:
                    nc.scalar.activation(
                        bias, mk, Copy, bias=-NEG_SCALE, scale=NEG_SCALE
                    )
                else:
                    nc.vector.tensor_scalar(
                        bias,
                        mk,
                        NEG_SCALE,
                        -NEG_SCALE,
                        mybir.AluOpType.mult,
                        mybir.AluOpType.add,
                    )

                eng = e_ps.tile([P, Lk], F32)
                for kc in range(KC):
                    ksl = slice(kc * 512, (kc + 1) * 512)
                    for hc in range(HB):
                        nc.tensor.matmul(
                            eng[:, ksl],
                            qwt_sb[:, hc, qsl],
                            kt_sb[:, hc, ksl],
                            start=(hc == 0),
                            stop=False,
                        )
                    nc.tensor.matmul(
                        eng[:, ksl], id16, bias[:, ksl], start=False, stop=True
                    )

                p16 = pp.tile([P, Lk], BF16)
                rsum = sums.tile([P, 1], F32, tag="rsum")
                nc.scalar.activation(p16, eng, Exp, accum_out=rsum)
                recip = sums.tile([P, 1], F32, tag="recip")
                nc.vector.reciprocal(recip, rsum)

                att = attp.tile([P, Lk], F32)
                nc.vector.tensor_scalar_mul(att, p16, recip)
                nc.scalar.dma_start(out=att_d[qsl, :], in_=att)

                pt = ptp.tile([P, LkB, P], BF16)
                for g4 in range(LkB // 4):
                    ps = pt_ps.tile([P, 512], BF16, tag="ptps")
                    for j in range(4):
                        kb = g4 * 4 + j
                        nc.tensor.transpose(
                            ps[:, j * P : (j + 1) * P],
                            p16[:, kb * P : (kb + 1) * P],
                            id16,
                        )
                    nc.vector.tensor_copy(pt[:, g4 * 4 : (g4 + 1) * 4, :], ps)

                ctx_sb = ctxp.tile([P, H], F32)
                for h2 in range(HC2):
                    hsl = slice(h2 * 512, (h2 + 1) * 512)
                    cps = c_ps.tile([P, 512], F32, tag="cps")
                    for kb in range(LkB):
                        nc.tensor.matmul(
                            cps,
                            pt[:, kb, :],
                            v_sb[:, kb, hsl],
                            start=(kb == 0),
                            stop=(kb == LkB - 1),
                        )
                    nc.vector.tensor_scalar_mul(ctx_sb[:, hsl], cps, recip)
                nc.scalar.dma_start(out=ctx_d[qsl, :], in_=ctx_sb)

    _split_wide_sync_waits(nc)
    return nc


_nc_cache = {}


def _get_nc(Lq, Lk, H):
    key = (Lq, Lk, H)
    if key not in _nc_cache:
        _nc_cache[key] = build_attention_nc(Lq, Lk, H)
    return _nc_cache[key]


def kernel(query, key, value, mask, W, trace=False):
    query = np.ascontiguousarray(np.asarray(query, dtype=np.float32))
    key = np.ascontiguousarray(np.asarray(key, dtype=np.float32))
    value = np.ascontiguousarray(np.asarray(value, dtype=np.float32))
    mask = np.ascontiguousarray(np.asarray(mask, dtype=np.int32))
    W = np.ascontiguousarray(np.asarray(W, dtype=np.float32))

    B, Lq, H = query.shape
    Lk = key.shape[1]
    assert B == 8, f"expected B=8, got {B}"

    nc = _get_nc(Lq, Lk, H)
    in_maps = [
        {
            "query": query[b],
            "key": key[b],
            "value": value[b],
            "mask": mask[b],
            "W": W,
        }
        for b in range(B)
    ]
    res = run_bass_kernel_spmd(
        nc, in_maps, core_ids=list(range(B)), trace=trace
    )
    context = np.stack([r["context"] for r in res.results])
    attention = np.stack([r["attention"] for r in res.results])
    if trace:
        kernel.last_exec_time_ns = res.exec_time_ns
        kernel.last_results = res
    return context, attention


# revision 12
# speedup vs baseline: 1.1744x; 1.0943x over previous
"""GeneralAttention Trainium2 Bass kernel.

Computes, for each batch b (data-parallel, one batch per NeuronCore):
    key_t   = key @ W^T
    energy  = (query @ key_t^T) / sqrt(H)        [B, Lq, Lk]
    energy  = where(mask == 0, -1e10, energy)
    att     = softmax(energy, axis=-1)
    context = att @ value                        [B, Lq, H]
returns (context, attention).

Math used on-chip (identical up to fp rounding):
    energy  = (query @ (W/sqrt(H))) @ key^T      (transform Q, not K)
    p       = exp(energy + (mask-1)*1e10)        (masked entries -> exp(-1e10) == 0)
    att     = exp(energy + bias - ln(sum(p)))    (no row-max needed: |energy| < ~10)
    context = (p @ value) * (1/sum(p))
The additive mask is injected directly into the PSUM energy accumulation with
one extra matmul whose stationary operand is the 128x128 identity and whose
moving operand is the bias tile, so no vector-engine pass over [128, Lk] is
needed before the exponent.
"""

import math
import sys
from contextlib import ExitStack

for _p in ("/opt/trn_rl_repo",):
    if _p not in sys.path:
        sys.path.insert(0, _p)

import numpy as np

import concourse.bass as bass
import concourse.mybir as mybir
import concourse.tile as tile
from concourse.bass_utils import run_bass_kernel_spmd
from concourse.masks import make_identity

P = 128
NEG_SCALE = 1.0e10
F32 = mybir.dt.float32
BF16 = mybir.dt.bfloat16
I32 = mybir.dt.int32
Copy = mybir.ActivationFunctionType.Copy
Exp = mybir.ActivationFunctionType.Exp
Ln = mybir.ActivationFunctionType.Ln


_DMA_INSTS = (
    mybir.InstDMACopy,
    mybir.InstDMA,
    mybir.InstDmaTransposeAnt,
    mybir.InstDMAGatherAnt,
    mybir.InstDMAScatterAddAnt,
)


def _split_wide_sync_waits(nc, cap=1):
    """walrus in this container rejects >1 sync-wait command per instruction.
    Move excess waits onto preceding single-wait nops on the same engine —
    semantically identical (the sequencer stalls either way), just encoded
    across several instructions.  Excess sem updates on non-DMA instructions
    move to a following nop (same engine, in-order completion).  DMA updates
    are never moved (they fire at transfer completion, a nop would not)."""
    n_wait = n_upd = 0
    for f in nc.m.functions:
        for blk in f.blocks:
            new_insts = []
            for inst in blk.instructions:
                si = inst.sync_info
                if si is not None and si.on_wait and len(si.on_wait) > cap:
                    waits = list(si.on_wait)
                    for w in waits[:-cap]:
                        nop = mybir.InstNoOp(
                            name=f"waitsplit-{nc.next_id()}",
                            ins=[],
                            outs=[],
                            engine=inst.engine,
                            sync_info=mybir.SyncInfo(on_wait=[w], on_update=[]),
                        )
                        new_insts.append(nop)
                        n_wait += 1
                    si.on_wait = waits[-cap:]
                new_insts.append(inst)
                if (
                    si is not None
                    and si.on_update
                    and len(si.on_update) > cap
                    and not isinstance(inst, _DMA_INSTS)
                ):
                    upds = list(si.on_update)
                    si.on_update = upds[:cap]
                    for u in upds[cap:]:
                        nop = mybir.InstNoOp(
                            name=f"updsplit-{nc.next_id()}",
                            ins=[],
                            outs=[],
                            engine=inst.engine,
                            sync_info=mybir.SyncInfo(on_wait=[], on_update=[u]),
                        )
                        new_insts.append(nop)
                        n_upd += 1
            blk.instructions = new_insts
    return n_wait, n_upd


def build_attention_nc(Lq, Lk, H, mask_bias_on_act=True):
    assert Lq % 512 == 0 and Lk % 512 == 0 and H % 512 == 0
    HB, LqB, LkB = H // P, Lq // P, Lk // P
    KC, QC, HC2 = Lk // 512, Lq // 512, H // 512
    scale = 1.0 / math.sqrt(H)

    nc = bass.Bass(trn_type="TRN2")
    q_d = nc.dram_tensor("query", [Lq, H], F32, kind="ExternalInput")
    k_d = nc.dram_tensor("key", [Lk, H], F32, kind="ExternalInput")
    v_d = nc.dram_tensor("value", [Lk, H], F32, kind="ExternalInput")
    m_d = nc.dram_tensor("mask", [Lq, Lk], I32, kind="ExternalInput")
    w_d = nc.dram_tensor("W", [H, H], F32, kind="ExternalInput")
    ctx_d = nc.dram_tensor("context", [Lq, H], F32, kind="ExternalOutput")
    att_d = nc.dram_tensor("attention", [Lq, Lk], F32, kind="ExternalOutput")

    with tile.TileContext(nc) as tc, ExitStack() as ctx:
        persist = ctx.enter_context(tc.tile_pool(name="persist", bufs=1))
        # W natural layout (o-partition, o-chunk, h), pre-scaled by 1/sqrt(H)
        w_sb = persist.tile([P, HB, H], BF16)
        # value natural layout (k-partition, k-block, h)
        v_sb = persist.tile([P, LkB, H], BF16)
        # key^T (h-partition, h-chunk, k)
        kt_sb = persist.tile([P, HB, Lk], BF16)
        # (Q @ W/sqrt(H))^T (h-partition, h-chunk, q)
        qwt_sb = persist.tile([P, HB, Lq], BF16)
        id32 = persist.tile([P, P], F32)
        id16 = persist.tile([P, P], BF16)
        make_identity(nc, id32)
        make_identity(nc, id16)

        # ---------------- prep: W, V, K^T, q^T -> QW^T ----------------
        with (
            tc.tile_pool(name="prep_loads", bufs=2) as loads,
            tc.tile_pool(name="prep_misc", bufs=2) as prep_misc,
            tc.tile_pool(name="prep_ps", bufs=2, space="PSUM") as prep_ps,
            tc.tile_pool(name="qw_ps", bufs=2, space="PSUM") as qw_ps,
        ):
            # DMA queues are FIFO per HWDGE engine, and emission order sets
            # the scheduler's tie-break priority — so load in critical-path
            # order: first K-group 0 and W and Q-group 0 (together they
            # unblock the first energy matmuls ~30us in), then the remaining
            # K and Q groups, then V (only needed for the first context
            # matmul much later).  Mask rides the scalar-engine queue.
            def k_group(g):
                kf = loads.tile([P, 4, H], F32, tag="ldf", name="kf")
                if g == 0:
                    # per-tile DMAs: transposes of tile j start as soon as
                    # its slice lands, shortening the cold-start ramp
                    for j in range(4):
                        nc.sync.dma_start(
                            out=kf[:, j, :],
                            in_=k_d[(g * 4 + j) * P : (g * 4 + j + 1) * P, :],
                        )
                else:
                    nc.sync.dma_start(
                        out=kf,
                        in_=k_d[g * 512 : (g + 1) * 512, :].rearrange(
                            "(j p) h -> p j h", p=P
                        ),
                    )
                for hc in range(HB):
                    ps = prep_ps.tile([P, 512], F32, tag="tps", name="ps")
                    for j in range(4):
                        nc.tensor.transpose(
                            ps[:, j * P : (j + 1) * P],
                            kf[:, j, hc * P : (hc + 1) * P],
                            id32,
                        )
                    nc.vector.tensor_copy(kt_sb[:, hc, g * 512 : (g + 1) * 512], ps)

            def q_group(g):
                qf = loads.tile([P, 4, H], F32, tag="ldf", name="qf")
                if g == 0:
                    for j in range(4):
                        nc.sync.dma_start(
                            out=qf[:, j, :],
                            in_=q_d[(g * 4 + j) * P : (g * 4 + j + 1) * P, :],
                        )
                else:
                    nc.sync.dma_start(
                        out=qf,
                        in_=q_d[g * 512 : (g + 1) * 512, :].rearrange(
                            "(j p) h -> p j h", p=P
                        ),
                    )
                qt = loads.tile([P, HB, 512], BF16, tag="qt", name="qt")
                for oc in range(HB):
                    ps = prep_ps.tile([P, 512], F32, tag="tps", name="ps")
                    for j in range(4):
                        nc.tensor.transpose(
                            ps[:, j * P : (j + 1) * P],
                            qf[:, j, oc * P : (oc + 1) * P],
                            id32,
                        )
                    nc.scalar.copy(qt[:, oc, :], ps)
                for hc in range(HB):
                    qw = qw_ps.tile([P, 512], F32, tag="qw", name="qw")
                    for oc in range(HB):
                        nc.tensor.matmul(
                            qw,
                            w_sb[:, oc, hc * P : (hc + 1) * P],
                            qt[:, oc, :],
                            start=(oc == 0),
                            stop=(oc == HB - 1),
                        )
                    nc.scalar.copy(qwt_sb[:, hc, g * 512 : (g + 1) * 512], qw)

            k_group(0)
            for oc in range(HB):
                wf = prep_misc.tile([P, H], F32, tag="wf")
                nc.sync.dma_start(out=wf, in_=w_d[oc * P : (oc + 1) * P, :])
                nc.scalar.activation(w_sb[:, oc, :], wf, Copy, scale=scale)
            q_group(0)
            for g in range(1, LkB // 4):
                k_group(g)
                q_group(g)
            for g in range(LkB // 4, QC):
                q_group(g)

            for kb in range(LkB):
                vf = prep_misc.tile([P, H], F32, tag="vf")
                nc.sync.dma_start(out=vf, in_=v_d[kb * P : (kb + 1) * P, :])
                nc.vector.tensor_copy(v_sb[:, kb, :], vf)

        # ---------------- main loop over q row-blocks ----------------
        # The energy row [P, Lk] is computed in two PSUM halves (2 banks
        # each, bufs=2): exp of half 0 runs on ACT while half 1's matmuls
        # stream on PE, and the half-0 banks recycle for the next block
        # without waiting on the whole row's exp.  pT transposes interleave
        # with the first h-chunk of context matmuls so the PE never sits on
        # a PSUM-evacuation wait.
        EH = Lk // 2          # elements per energy half
        KC2 = max(1, KC // 2)  # 512-chunks per half
        with (
            tc.tile_pool(name="maskp", bufs=2) as maskp,
            tc.tile_pool(name="biasp", bufs=2) as biasp,
            tc.tile_pool(name="pp", bufs=2) as pp,
            tc.tile_pool(name="attp", bufs=2) as attp,
            tc.tile_pool(name="ptp", bufs=2) as ptp,
            tc.tile_pool(name="ctxp", bufs=2) as ctxp,
            tc.tile_pool(name="sums", bufs=3) as sums,
            tc.tile_pool(name="e_ps", bufs=2, space="PSUM") as e_ps,
            tc.tile_pool(name="pt_ps", bufs=1, space="PSUM") as pt_ps,
            tc.tile_pool(name="c_ps", bufs=1, space="PSUM") as c_ps,
        ):
            for qb in range(LqB):
                qsl = slice(qb * P, (qb + 1) * P)
                mk = maskp.tile([P, Lk], I32)
                nc.scalar.dma_start(out=mk, in_=m_d[qsl, :])
                bias = biasp.tile([P, Lk], BF16)
                if mask_bias_on_act:
                    nc.scalar.activation(
                        bias, mk, Copy, bias=-NEG_SCALE, scale=NEG_SCALE
                    )
                else:
                    nc.vector.tensor_scalar(
                        bias,
                        mk,
                        NEG_SCALE,
                        -NEG_SCALE,
                        mybir.AluOpType.mult,
                        mybir.AluOpType.add,
                    )

                p16 = pp.tile([P, Lk], BF16)
                halves = []
                for h in range(2 if KC > 1 else 1):
                    eng = e_ps.tile([P, EH if KC > 1 else Lk], F32, tag="eng")
                    for kc2 in range(KC2):
                        kc = h * KC2 + kc2
                        ksl = slice(kc * 512, (kc + 1) * 512)
                        esl = slice(kc2 * 512, (kc2 + 1) * 512)
                        for hc in range(HB):
                            nc.tensor.matmul(
                                eng[:, esl],
                                qwt_sb[:, hc, qsl],
                                kt_sb[:, hc, ksl],
                                start=(hc == 0),
                                stop=False,
                            )
                        nc.tensor.matmul(
                            eng[:, esl], id16, bias[:, ksl], start=False, stop=True
                        )
                    rs = sums.tile([P, 1], F32, tag=f"rsum{h}", name="rs")
                    nc.scalar.activation(
                        p16[:, h * EH : h * EH + eng.shape[-1]], eng, Exp,
                        accum_out=rs,
                    )
                    halves.append(rs)

                rsum = sums.tile([P, 1], F32, tag="rsum")
                if len(halves) == 2:
                    nc.vector.tensor_add(rsum, halves[0], halves[1])
                else:
                    nc.vector.tensor_copy(rsum, halves[0])
                recip = sums.tile([P, 1], F32, tag="recip")
                nc.vector.reciprocal(recip, rsum)

                att = attp.tile([P, Lk], F32)
                nc.vector.tensor_scalar_mul(att, p16, recip)
                nc.scalar.dma_start(out=att_d[qsl, :], in_=att)

                # pT transposes interleaved with context h-chunk 0 matmuls
                pt = ptp.tile([P, LkB, P], BF16)
                cps0 = c_ps.tile([P, 512], F32, tag="cps", name="cps0")
                for g4 in range(LkB // 4):
                    ps = pt_ps.tile([P, 512], BF16, tag="ptps")
                    for j in range(4):
                        kb = g4 * 4 + j
                        nc.tensor.transpose(
                            ps[:, j * P : (j + 1) * P],
                            p16[:, kb * P : (kb + 1) * P],
                            id16,
                        )
                    nc.vector.tensor_copy(pt[:, g4 * 4 : (g4 + 1) * 4, :], ps)
                    for kb in range(g4 * 4, g4 * 4 + 4):
                        nc.tensor.matmul(
                            cps0,
                            pt[:, kb, :],
                            v_sb[:, kb, 0:512],
                            start=(kb == 0),
                            stop=(kb == LkB - 1),
                        )

                ctx_sb = ctxp.tile([P, H], F32)
                nc.vector.tensor_scalar_mul(ctx_sb[:, 0:512], cps0, recip)
                for h2 in range(1, HC2):
                    hsl = slice(h2 * 512, (h2 + 1) * 512)
                    cps = c_ps.tile([P, 512], F32, tag="cps", name="cps")
                    for kb in range(LkB):
                        nc.tensor.matmul(
                            cps,
                            pt[:, kb, :],
                            v_sb[:, kb, hsl],
                            start=(kb == 0),
                            stop=(kb == LkB - 1),
                        )
                    nc.vector.tensor_scalar_mul(ctx_sb[:, hsl], cps, recip)
                nc.scalar.dma_start(out=ctx_d[qsl, :], in_=ctx_sb)

    _split_wide_sync_waits(nc)
    return nc


_nc_cache = {}


def _get_nc(Lq, Lk, H):
    key = (Lq, Lk, H)
    if key not in _nc_cache:
        _nc_cache[key] = build_attention_nc(Lq, Lk, H)
    return _nc_cache[key]


def kernel(query, key, value, mask, W, trace=False):
    query = np.ascontiguousarray(np.asarray(query, dtype=np.float32))
    key = np.ascontiguousarray(np.asarray(key, dtype=np.float32))
    value = np.ascontiguousarray(np.asarray(value, dtype=np.float32))
    mask = np.ascontiguousarray(np.asarray(mask, dtype=np.int32))
    W = np.ascontiguousarray(np.asarray(W, dtype=np.float32))

    B, Lq, H = query.shape
    Lk = key.shape[1]
    assert B == 8, f"expected B=8, got {B}"

    nc = _get_nc(Lq, Lk, H)
    in_maps = [
        {
            "query": query[b],
            "key": key[b],
            "value": value[b],
            "mask": mask[b],
            "W": W,
        }
        for b in range(B)
    ]
    res = run_bass_kernel_spmd(
        nc, in_maps, core_ids=list(range(B)), trace=trace
    )
    context = np.stack([r["context"] for r in res.results])
    attention = np.stack([r["attention"] for r in res.results])
    if trace:
        kernel.last_exec_time_ns = res.exec_time_ns
        kernel.last_results = res
    return context, attention


# revision 13
# speedup vs baseline: 1.1948x; 1.0173x over previous
"""GeneralAttention Trainium2 Bass kernel.

Computes, for each batch b (data-parallel, one batch per NeuronCore):
    key_t   = key @ W^T
    energy  = (query @ key_t^T) / sqrt(H)        [B, Lq, Lk]
    energy  = where(mask == 0, -1e10, energy)
    att     = softmax(energy, axis=-1)
    context = att @ value                        [B, Lq, H]
returns (context, attention).

Math used on-chip (identical up to fp rounding):
    energy  = (query @ (W/sqrt(H))) @ key^T      (transform Q, not K)
    p       = exp(energy + (mask-1)*1e10)        (masked entries -> exp(-1e10) == 0)
    att     = exp(energy + bias - ln(sum(p)))    (no row-max needed: |energy| < ~10)
    context = (p @ value) * (1/sum(p))
The additive mask is injected directly into the PSUM energy accumulation with
one extra matmul whose stationary operand is the 128x128 identity and whose
moving operand is the bias tile, so no vector-engine pass over [128, Lk] is
needed before the exponent.
"""

import math
import sys
from contextlib import ExitStack

for _p in ("/opt/trn_rl_repo",):
    if _p not in sys.path:
        sys.path.insert(0, _p)

import numpy as np

import concourse.bass as bass
import concourse.mybir as mybir
import concourse.tile as tile
from concourse.bass_utils import run_bass_kernel_spmd
from concourse.masks import make_identity

P = 128
NEG_SCALE = 1.0e10
F32 = mybir.dt.float32
BF16 = mybir.dt.bfloat16
I32 = mybir.dt.int32
Copy = mybir.ActivationFunctionType.Copy
Exp = mybir.ActivationFunctionType.Exp
Ln = mybir.ActivationFunctionType.Ln


_DMA_INSTS = (
    mybir.InstDMACopy,
    mybir.InstDMA,
    mybir.InstDmaTransposeAnt,
    mybir.InstDMAGatherAnt,
    mybir.InstDMAScatterAddAnt,
)


def _split_wide_sync_waits(nc, cap=1):
    """walrus in this container rejects >1 sync-wait command per instruction.
    Move excess waits onto preceding single-wait nops on the same engine —
    semantically identical (the sequencer stalls either way), just encoded
    across several instructions.  Excess sem updates on non-DMA instructions
    move to a following nop (same engine, in-order completion).  DMA updates
    are never moved (they fire at transfer completion, a nop would not)."""
    n_wait = n_upd = 0
    for f in nc.m.functions:
        for blk in f.blocks:
            new_insts = []
            for inst in blk.instructions:
                si = inst.sync_info
                if si is not None and si.on_wait and len(si.on_wait) > cap:
                    waits = list(si.on_wait)
                    for w in waits[:-cap]:
                        nop = mybir.InstNoOp(
                            name=f"waitsplit-{nc.next_id()}",
                            ins=[],
                            outs=[],
                            engine=inst.engine,
                            sync_info=mybir.SyncInfo(on_wait=[w], on_update=[]),
                        )
                        new_insts.append(nop)
                        n_wait += 1
                    si.on_wait = waits[-cap:]
                new_insts.append(inst)
                if (
                    si is not None
                    and si.on_update
                    and len(si.on_update) > cap
                    and not isinstance(inst, _DMA_INSTS)
                ):
                    upds = list(si.on_update)
                    si.on_update = upds[:cap]
                    for u in upds[cap:]:
                        nop = mybir.InstNoOp(
                            name=f"updsplit-{nc.next_id()}",
                            ins=[],
                            outs=[],
                            engine=inst.engine,
                            sync_info=mybir.SyncInfo(on_wait=[], on_update=[u]),
                        )
                        new_insts.append(nop)
                        n_upd += 1
            blk.instructions = new_insts
    return n_wait, n_upd


def build_attention_nc(Lq, Lk, H, mask_bias_on_act=True):
    assert Lq % 512 == 0 and Lk % 512 == 0 and H % 512 == 0
    HB, LqB, LkB = H // P, Lq // P, Lk // P
    KC, QC, HC2 = Lk // 512, Lq // 512, H // 512
    scale = 1.0 / math.sqrt(H)

    nc = bass.Bass(trn_type="TRN2")
    q_d = nc.dram_tensor("query", [Lq, H], F32, kind="ExternalInput")
    k_d = nc.dram_tensor("key", [Lk, H], F32, kind="ExternalInput")
    v_d = nc.dram_tensor("value", [Lk, H], F32, kind="ExternalInput")
    m_d = nc.dram_tensor("mask", [Lq, Lk], I32, kind="ExternalInput")
    w_d = nc.dram_tensor("W", [H, H], F32, kind="ExternalInput")
    ctx_d = nc.dram_tensor("context", [Lq, H], F32, kind="ExternalOutput")
    att_d = nc.dram_tensor("attention", [Lq, Lk], F32, kind="ExternalOutput")

    with tile.TileContext(nc) as tc, ExitStack() as ctx:
        persist = ctx.enter_context(tc.tile_pool(name="persist", bufs=1))
        # W natural layout (o-partition, o-chunk, h), pre-scaled by 1/sqrt(H)
        w_sb = persist.tile([P, HB, H], BF16)
        # value natural layout (k-partition, k-block, h)
        v_sb = persist.tile([P, LkB, H], BF16)
        # key^T (h-partition, h-chunk, k)
        kt_sb = persist.tile([P, HB, Lk], BF16)
        # (Q @ W/sqrt(H))^T (h-partition, h-chunk, q)
        qwt_sb = persist.tile([P, HB, Lq], BF16)
        id16 = persist.tile([P, P], BF16)
        make_identity(nc, id16)

        # ---------------- prep: W, V, K^T, q^T -> QW^T ----------------
        with (
            tc.tile_pool(name="prep_loads", bufs=2) as loads,
            tc.tile_pool(name="prep_misc", bufs=2) as prep_misc,
            tc.tile_pool(name="prep_ps", bufs=2, space="PSUM") as prep_ps,
            tc.tile_pool(name="qw_ps", bufs=2, space="PSUM") as qw_ps,
        ):
            # DMA queues are FIFO per HWDGE engine, and emission order sets
            # the scheduler's tie-break priority — so load in critical-path
            # order: first K-group 0 and W and Q-group 0 (together they
            # unblock the first energy matmuls ~30us in), then the remaining
            # K and Q groups, then V (only needed for the first context
            # matmul much later).  Mask rides the scalar-engine queue.
            def k_group(g):
                kf = loads.tile([P, 4, H], F32, tag="ldf", name="kf")
                if g == 0:
                    # per-tile DMAs: transposes of tile j start as soon as
                    # its slice lands, shortening the cold-start ramp
                    for j in range(4):
                        nc.sync.dma_start(
                            out=kf[:, j, :],
                            in_=k_d[(g * 4 + j) * P : (g * 4 + j + 1) * P, :],
                        )
                else:
                    nc.sync.dma_start(
                        out=kf,
                        in_=k_d[g * 512 : (g + 1) * 512, :].rearrange(
                            "(j p) h -> p j h", p=P
                        ),
                    )
                kb16 = loads.tile([P, 4, H], BF16, tag="ld16", name="kb16")
                nc.vector.tensor_copy(kb16, kf)
                for hc in range(HB):
                    ps = prep_ps.tile([P, 512], BF16, tag="tps", name="ps")
                    for j in range(4):
                        nc.tensor.transpose(
                            ps[:, j * P : (j + 1) * P],
                            kb16[:, j, hc * P : (hc + 1) * P],
                            id16,
                        )
                    nc.vector.tensor_copy(kt_sb[:, hc, g * 512 : (g + 1) * 512], ps)

            def q_group(g):
                qf = loads.tile([P, 4, H], F32, tag="ldf", name="qf")
                if g == 0:
                    for j in range(4):
                        nc.sync.dma_start(
                            out=qf[:, j, :],
                            in_=q_d[(g * 4 + j) * P : (g * 4 + j + 1) * P, :],
                        )
                else:
                    nc.sync.dma_start(
                        out=qf,
                        in_=q_d[g * 512 : (g + 1) * 512, :].rearrange(
                            "(j p) h -> p j h", p=P
                        ),
                    )
                qb16 = loads.tile([P, 4, H], BF16, tag="ld16", name="qb16")
                nc.scalar.copy(qb16, qf)
                qt = loads.tile([P, HB, 512], BF16, tag="qt", name="qt")
                for oc in range(HB):
                    ps = prep_ps.tile([P, 512], BF16, tag="tps", name="ps")
                    for j in range(4):
                        nc.tensor.transpose(
                            ps[:, j * P : (j + 1) * P],
                            qb16[:, j, oc * P : (oc + 1) * P],
                            id16,
                        )
                    nc.scalar.copy(qt[:, oc, :], ps)
                for hc in range(HB):
                    qw = qw_ps.tile([P, 512], F32, tag="qw", name="qw")
                    for oc in range(HB):
                        nc.tensor.matmul(
                            qw,
                            w_sb[:, oc, hc * P : (hc + 1) * P],
                            qt[:, oc, :],
                            start=(oc == 0),
                            stop=(oc == HB - 1),
                        )
                    nc.scalar.copy(qwt_sb[:, hc, g * 512 : (g + 1) * 512], qw)

            k_group(0)
            for oc in range(HB):
                wf = prep_misc.tile([P, H], F32, tag="wf")
                nc.sync.dma_start(out=wf, in_=w_d[oc * P : (oc + 1) * P, :])
                nc.scalar.activation(w_sb[:, oc, :], wf, Copy, scale=scale)
            q_group(0)
            for g in range(1, LkB // 4):
                k_group(g)
                q_group(g)
            for g in range(LkB // 4, QC):
                q_group(g)

            for kb in range(LkB):
                vf = prep_misc.tile([P, H], F32, tag="vf")
                nc.sync.dma_start(out=vf, in_=v_d[kb * P : (kb + 1) * P, :])
                nc.vector.tensor_copy(v_sb[:, kb, :], vf)

        # ---------------- main loop over q row-blocks ----------------
        # The energy row [P, Lk] is computed in two PSUM halves (2 banks
        # each, bufs=2): exp of half 0 runs on ACT while half 1's matmuls
        # stream on PE, and the half-0 banks recycle for the next block
        # without waiting on the whole row's exp.  pT transposes interleave
        # with the first h-chunk of context matmuls so the PE never sits on
        # a PSUM-evacuation wait.
        EH = Lk // 2          # elements per energy half
        KC2 = max(1, KC // 2)  # 512-chunks per half
        with (
            tc.tile_pool(name="maskp", bufs=2) as maskp,
            tc.tile_pool(name="biasp", bufs=2) as biasp,
            tc.tile_pool(name="pp", bufs=2) as pp,
            tc.tile_pool(name="attp", bufs=2) as attp,
            tc.tile_pool(name="ptp", bufs=2) as ptp,
            tc.tile_pool(name="ctxp", bufs=2) as ctxp,
            tc.tile_pool(name="sums", bufs=3) as sums,
            tc.tile_pool(name="e_ps", bufs=2, space="PSUM") as e_ps,
            tc.tile_pool(name="pt_ps", bufs=1, space="PSUM") as pt_ps,
            tc.tile_pool(name="c_ps", bufs=1, space="PSUM") as c_ps,
        ):
            for qb in range(LqB):
                qsl = slice(qb * P, (qb + 1) * P)
                mk = maskp.tile([P, Lk], I32)
                nc.scalar.dma_start(out=mk, in_=m_d[qsl, :])
                bias = biasp.tile([P, Lk], BF16)
                if mask_bias_on_act:
                    nc.scalar.activation(
                        bias, mk, Copy, bias=-NEG_SCALE, scale=NEG_SCALE
                    )
                else:
                    nc.vector.tensor_scalar(
                        bias,
                        mk,
                        NEG_SCALE,
                        -NEG_SCALE,
                        mybir.AluOpType.mult,
                        mybir.AluOpType.add,
                    )

                p16 = pp.tile([P, Lk], BF16)
                halves = []
                for h in range(2 if KC > 1 else 1):
                    eng = e_ps.tile([P, EH if KC > 1 else Lk], F32, tag="eng")
                    for kc2 in range(KC2):
                        kc = h * KC2 + kc2
                        ksl = slice(kc * 512, (kc + 1) * 512)
                        esl = slice(kc2 * 512, (kc2 + 1) * 512)
                        for hc in range(HB):
                            nc.tensor.matmul(
                                eng[:, esl],
                                qwt_sb[:, hc, qsl],
                                kt_sb[:, hc, ksl],
                                start=(hc == 0),
                                stop=False,
                            )
                        nc.tensor.matmul(
                            eng[:, esl], id16, bias[:, ksl], start=False, stop=True
                        )
                    rs = sums.tile([P, 1], F32, tag=f"rsum{h}", name="rs")
                    nc.scalar.activation(
                        p16[:, h * EH : h * EH + eng.shape[-1]], eng, Exp,
                        accum_out=rs,
                    )
                    halves.append(rs)

                rsum = sums.tile([P, 1], F32, tag="rsum")
                if len(halves) == 2:
                    nc.vector.tensor_add(rsum, halves[0], halves[1])
                else:
                    nc.vector.tensor_copy(rsum, halves[0])
                recip = sums.tile([P, 1], F32, tag="recip")
                nc.vector.reciprocal(recip, rsum)

                att = attp.tile([P, Lk], F32)
                nc.vector.tensor_scalar_mul(att, p16, recip)
                nc.scalar.dma_start(out=att_d[qsl, :], in_=att)

                # pT transposes interleaved with context h-chunk 0 matmuls
                pt = ptp.tile([P, LkB, P], BF16)
                cps0 = c_ps.tile([P, 512], F32, tag="cps", name="cps0")
                for g4 in range(LkB // 4):
                    ps = pt_ps.tile([P, 512], BF16, tag="ptps")
                    for j in range(4):
                        kb = g4 * 4 + j
                        nc.tensor.transpose(
                            ps[:, j * P : (j + 1) * P],
                            p16[:, kb * P : (kb + 1) * P],
                            id16,
                        )
                    nc.vector.tensor_copy(pt[:, g4 * 4 : (g4 + 1) * 4, :], ps)
                    for kb in range(g4 * 4, g4 * 4 + 4):
                        nc.tensor.matmul(
                            cps0,
                            pt[:, kb, :],
                            v_sb[:, kb, 0:512],
                            start=(kb == 0),
                            stop=(kb == LkB - 1),
                        )

                ctx_sb = ctxp.tile([P, H], F32)
                nc.vector.tensor_scalar_mul(ctx_sb[:, 0:512], cps0, recip)
                for h2 in range(1, HC2):
                    hsl = slice(h2 * 512, (h2 + 1) * 512)
                    cps = c_ps.tile([P, 512], F32, tag="cps", name="cps")
                    for kb in range(LkB):
                        nc.tensor.matmul(
                            cps,
                            pt[:, kb, :],
                            v_sb[:, kb, hsl],
                            start=(kb == 0),
                            stop=(kb == LkB - 1),
                        )
                    nc.vector.tensor_scalar_mul(ctx_sb[:, hsl], cps, recip)
                nc.scalar.dma_start(out=ctx_d[qsl, :], in_=ctx_sb)

    _split_wide_sync_waits(nc)
    return nc


_nc_cache = {}


def _get_nc(Lq, Lk, H):
    key = (Lq, Lk, H)
    if key not in _nc_cache:
        _nc_cache[key] = build_attention_nc(Lq, Lk, H)
    return _nc_cache[key]


def kernel(query, key, value, mask, W, trace=False):
    query = np.ascontiguousarray(np.asarray(query, dtype=np.float32))
    key = np.ascontiguousarray(np.asarray(key, dtype=np.float32))
    value = np.ascontiguousarray(np.asarray(value, dtype=np.float32))
    mask = np.ascontiguousarray(np.asarray(mask, dtype=np.int32))
    W = np.ascontiguousarray(np.asarray(W, dtype=np.float32))

    B, Lq, H = query.shape
    Lk = key.shape[1]
    assert B == 8, f"expected B=8, got {B}"

    nc = _get_nc(Lq, Lk, H)
    in_maps = [
        {
            "query": query[b],
            "key": key[b],
            "value": value[b],
            "mask": mask[b],
            "W": W,
        }
        for b in range(B)
    ]
    res = run_bass_kernel_spmd(
        nc, in_maps, core_ids=list(range(B)), trace=trace
    )
    context = np.stack([r["context"] for r in res.results])
    attention = np.stack([r["attention"] for r in res.results])
    if trace:
        kernel.last_exec_time_ns = res.exec_time_ns
        kernel.last_results = res
    return context, attention


# revision 14
# speedup vs baseline: 1.2095x; 1.0124x over previous
"""GeneralAttention Trainium2 Bass kernel.

Computes, for each batch b (data-parallel, one batch per NeuronCore):
    key_t   = key @ W^T
    energy  = (query @ key_t^T) / sqrt(H)        [B, Lq, Lk]
    energy  = where(mask == 0, -1e10, energy)
    att     = softmax(energy, axis=-1)
    context = att @ value                        [B, Lq, H]
returns (context, attention).

Math used on-chip (identical up to fp rounding):
    energy  = (query @ (W/sqrt(H))) @ key^T      (transform Q, not K)
    p       = exp(energy + (mask-1)*1e10)        (masked entries -> exp(-1e10) == 0)
    att     = exp(energy + bias - ln(sum(p)))    (no row-max needed: |energy| < ~10)
    context = (p @ value) * (1/sum(p))
The additive mask is injected directly into the PSUM energy accumulation with
one extra matmul whose stationary operand is the 128x128 identity and whose
moving operand is the bias tile, so no vector-engine pass over [128, Lk] is
needed before the exponent.
"""

import math
import sys
from contextlib import ExitStack

for _p in ("/opt/trn_rl_repo",):
    if _p not in sys.path:
        sys.path.insert(0, _p)

import numpy as np

import concourse.bass as bass
import concourse.mybir as mybir
import concourse.tile as tile
from concourse.bass_utils import run_bass_kernel_spmd
from concourse.masks import make_identity

P = 128
NEG_SCALE = 1.0e10
F32 = mybir.dt.float32
BF16 = mybir.dt.bfloat16
I32 = mybir.dt.int32
Copy = mybir.ActivationFunctionType.Copy
Exp = mybir.ActivationFunctionType.Exp
Ln = mybir.ActivationFunctionType.Ln


_DMA_INSTS = (
    mybir.InstDMACopy,
    mybir.InstDMA,
    mybir.InstDmaTransposeAnt,
    mybir.InstDMAGatherAnt,
    mybir.InstDMAScatterAddAnt,
)


def _split_wide_sync_waits(nc, cap=1):
    """walrus in this container rejects >1 sync-wait command per instruction.
    Move excess waits onto preceding single-wait nops on the same engine —
    semantically identical (the sequencer stalls either way), just encoded
    across several instructions.  Excess sem updates on non-DMA instructions
    move to a following nop (same engine, in-order completion).  DMA updates
    are never moved (they fire at transfer completion, a nop would not)."""
    n_wait = n_upd = 0
    for f in nc.m.functions:
        for blk in f.blocks:
            new_insts = []
            for inst in blk.instructions:
                si = inst.sync_info
                if si is not None and si.on_wait and len(si.on_wait) > cap:
                    waits = list(si.on_wait)
                    for w in waits[:-cap]:
                        nop = mybir.InstNoOp(
                            name=f"waitsplit-{nc.next_id()}",
                            ins=[],
                            outs=[],
                            engine=inst.engine,
                            sync_info=mybir.SyncInfo(on_wait=[w], on_update=[]),
                        )
                        new_insts.append(nop)
                        n_wait += 1
                    si.on_wait = waits[-cap:]
                new_insts.append(inst)
                if (
                    si is not None
                    and si.on_update
                    and len(si.on_update) > cap
                    and not isinstance(inst, _DMA_INSTS)
                ):
                    upds = list(si.on_update)
                    si.on_update = upds[:cap]
                    for u in upds[cap:]:
                        nop = mybir.InstNoOp(
                            name=f"updsplit-{nc.next_id()}",
                            ins=[],
                            outs=[],
                            engine=inst.engine,
                            sync_info=mybir.SyncInfo(on_wait=[], on_update=[u]),
                        )
                        new_insts.append(nop)
                        n_upd += 1
            blk.instructions = new_insts
    return n_wait, n_upd


def build_attention_nc(Lq, Lk, H, mask_bias_on_act=True):
    assert Lq % 512 == 0 and Lk % 512 == 0 and H % 512 == 0
    HB, LqB, LkB = H // P, Lq // P, Lk // P
    KC, QC, HC2 = Lk // 512, Lq // 512, H // 512
    scale = 1.0 / math.sqrt(H)

    nc = bass.Bass(trn_type="TRN2")
    q_d = nc.dram_tensor("query", [Lq, H], F32, kind="ExternalInput")
    k_d = nc.dram_tensor("key", [Lk, H], F32, kind="ExternalInput")
    v_d = nc.dram_tensor("value", [Lk, H], F32, kind="ExternalInput")
    m_d = nc.dram_tensor("mask", [Lq, Lk], I32, kind="ExternalInput")
    w_d = nc.dram_tensor("W", [H, H], F32, kind="ExternalInput")
    ctx_d = nc.dram_tensor("context", [Lq, H], F32, kind="ExternalOutput")
    att_d = nc.dram_tensor("attention", [Lq, Lk], F32, kind="ExternalOutput")

    with tile.TileContext(nc) as tc, ExitStack() as ctx:
        persist = ctx.enter_context(tc.tile_pool(name="persist", bufs=1))
        # W natural layout (o-partition, o-chunk, h), pre-scaled by 1/sqrt(H)
        w_sb = persist.tile([P, HB, H], BF16)
        # value natural layout (k-partition, k-block, h)
        v_sb = persist.tile([P, LkB, H], BF16)
        # key^T (h-partition, h-chunk, k)
        kt_sb = persist.tile([P, HB, Lk], BF16)
        # (Q @ W/sqrt(H))^T (h-partition, h-chunk, q)
        qwt_sb = persist.tile([P, HB, Lq], BF16)
        id16 = persist.tile([P, P], BF16)
        make_identity(nc, id16)

        # flat PSUM pools shared by prep and main phases (same tags reuse
        # the same banks with no cross-phase handoff serialization):
        # energy 2x2 banks, transposes 2 banks, QWT/context 2 banks = 8.
        e_ps = ctx.enter_context(tc.tile_pool(name="e_ps", bufs=2, space="PSUM"))
        t_ps = ctx.enter_context(tc.tile_pool(name="t_ps", bufs=2, space="PSUM"))
        cq_ps = ctx.enter_context(tc.tile_pool(name="cq_ps", bufs=2, space="PSUM"))

        # ---------------- prep: W, V, K^T, q^T -> QW^T ----------------
        with (
            tc.tile_pool(name="prep_loads", bufs=2) as loads,
            tc.tile_pool(name="prep_misc", bufs=2) as prep_misc,
        ):
            # DMA queues are FIFO per HWDGE engine, and emission order sets
            # the scheduler's tie-break priority — so load in critical-path
            # order: first K-group 0 and W and Q-group 0 (together they
            # unblock the first energy matmuls ~30us in), then the remaining
            # K and Q groups, then V (only needed for the first context
            # matmul much later).  Mask rides the scalar-engine queue.
            def k_group(g):
                kf = loads.tile([P, 4, H], F32, tag="ldf", name="kf")
                if g == 0:
                    # per-tile DMAs: transposes of tile j start as soon as
                    # its slice lands, shortening the cold-start ramp
                    for j in range(4):
                        nc.sync.dma_start(
                            out=kf[:, j, :],
                            in_=k_d[(g * 4 + j) * P : (g * 4 + j + 1) * P, :],
                        )
                else:
                    nc.sync.dma_start(
                        out=kf,
                        in_=k_d[g * 512 : (g + 1) * 512, :].rearrange(
                            "(j p) h -> p j h", p=P
                        ),
                    )
                kb16 = loads.tile([P, 4, H], BF16, tag="ld16", name="kb16")
                nc.vector.tensor_copy(kb16, kf)
                for hc in range(HB):
                    ps = t_ps.tile([P, 512], BF16, tag="tps", name="ps")
                    for j in range(4):
                        nc.tensor.transpose(
                            ps[:, j * P : (j + 1) * P],
                            kb16[:, j, hc * P : (hc + 1) * P],
                            id16,
                        )
                    nc.vector.tensor_copy(kt_sb[:, hc, g * 512 : (g + 1) * 512], ps)

            def q_group(g):
                qf = loads.tile([P, 4, H], F32, tag="ldf", name="qf")
                if g == 0:
                    for j in range(4):
                        nc.sync.dma_start(
                            out=qf[:, j, :],
                            in_=q_d[(g * 4 + j) * P : (g * 4 + j + 1) * P, :],
                        )
                else:
                    nc.sync.dma_start(
                        out=qf,
                        in_=q_d[g * 512 : (g + 1) * 512, :].rearrange(
                            "(j p) h -> p j h", p=P
                        ),
                    )
                qb16 = loads.tile([P, 4, H], BF16, tag="ld16", name="qb16")
                nc.scalar.copy(qb16, qf)
                qt = loads.tile([P, HB, 512], BF16, tag="qt", name="qt")
                for oc in range(HB):
                    ps = t_ps.tile([P, 512], BF16, tag="tps", name="ps")
                    for j in range(4):
                        nc.tensor.transpose(
                            ps[:, j * P : (j + 1) * P],
                            qb16[:, j, oc * P : (oc + 1) * P],
                            id16,
                        )
                    nc.scalar.copy(qt[:, oc, :], ps)
                for hc in range(HB):
                    qw = cq_ps.tile([P, 512], F32, tag="cq", name="qw")
                    for oc in range(HB):
                        nc.tensor.matmul(
                            qw,
                            w_sb[:, oc, hc * P : (hc + 1) * P],
                            qt[:, oc, :],
                            start=(oc == 0),
                            stop=(oc == HB - 1),
                        )
                    nc.scalar.copy(qwt_sb[:, hc, g * 512 : (g + 1) * 512], qw)

            k_group(0)
            for oc in range(HB):
                wf = prep_misc.tile([P, H], F32, tag="wf")
                nc.sync.dma_start(out=wf, in_=w_d[oc * P : (oc + 1) * P, :])
                nc.scalar.activation(w_sb[:, oc, :], wf, Copy, scale=scale)
            q_group(0)
            for g in range(1, LkB // 4):
                k_group(g)
                q_group(g)
            for g in range(LkB // 4, QC):
                q_group(g)

            for kb in range(LkB):
                vf = prep_misc.tile([P, H], F32, tag="vf")
                nc.sync.dma_start(out=vf, in_=v_d[kb * P : (kb + 1) * P, :])
                nc.vector.tensor_copy(v_sb[:, kb, :], vf)

        # ---------------- main loop over q row-blocks ----------------
        # The energy row [P, Lk] is computed in two PSUM halves (2 banks
        # each, bufs=2): exp of half 0 runs on ACT while half 1's matmuls
        # stream on PE, and the half-0 banks recycle for the next block
        # without waiting on the whole row's exp.  pT transposes interleave
        # with the first h-chunk of context matmuls so the PE never sits on
        # a PSUM-evacuation wait.
        EH = Lk // 2          # elements per energy half
        KC2 = max(1, KC // 2)  # 512-chunks per half
        with (
            tc.tile_pool(name="maskp", bufs=2) as maskp,
            tc.tile_pool(name="biasp", bufs=2) as biasp,
            tc.tile_pool(name="pp", bufs=2) as pp,
            tc.tile_pool(name="attp", bufs=2) as attp,
            tc.tile_pool(name="ptp", bufs=2) as ptp,
            tc.tile_pool(name="ctxp", bufs=2) as ctxp,
            tc.tile_pool(name="sums", bufs=3) as sums,
        ):
            for qb in range(LqB):
                qsl = slice(qb * P, (qb + 1) * P)
                mk = maskp.tile([P, Lk], I32)
                nc.scalar.dma_start(out=mk, in_=m_d[qsl, :])
                bias = biasp.tile([P, Lk], BF16)
                if mask_bias_on_act:
                    nc.scalar.activation(
                        bias, mk, Copy, bias=-NEG_SCALE, scale=NEG_SCALE
                    )
                else:
                    nc.vector.tensor_scalar(
                        bias,
                        mk,
                        NEG_SCALE,
                        -NEG_SCALE,
                        mybir.AluOpType.mult,
                        mybir.AluOpType.add,
                    )

                p16 = pp.tile([P, Lk], BF16)
                halves = []
                for h in range(2 if KC > 1 else 1):
                    eng = e_ps.tile([P, EH if KC > 1 else Lk], F32, tag="eng")
                    for kc2 in range(KC2):
                        kc = h * KC2 + kc2
                        ksl = slice(kc * 512, (kc + 1) * 512)
                        esl = slice(kc2 * 512, (kc2 + 1) * 512)
                        for hc in range(HB):
                            nc.tensor.matmul(
                                eng[:, esl],
                                qwt_sb[:, hc, qsl],
                                kt_sb[:, hc, ksl],
                                start=(hc == 0),
                                stop=False,
                            )
                        nc.tensor.matmul(
                            eng[:, esl], id16, bias[:, ksl], start=False, stop=True
                        )
                    rs = sums.tile([P, 1], F32, tag=f"rsum{h}", name="rs")
                    nc.scalar.activation(
                        p16[:, h * EH : h * EH + eng.shape[-1]], eng, Exp,
                        accum_out=rs,
                    )
                    halves.append(rs)

                rsum = sums.tile([P, 1], F32, tag="rsum")
                if len(halves) == 2:
                    nc.vector.tensor_add(rsum, halves[0], halves[1])
                else:
                    nc.vector.tensor_copy(rsum, halves[0])
                recip = sums.tile([P, 1], F32, tag="recip")
                nc.vector.reciprocal(recip, rsum)

                att = attp.tile([P, Lk], F32)
                nc.vector.tensor_scalar_mul(att, p16, recip)
                nc.scalar.dma_start(out=att_d[qsl, :], in_=att)

                # pT transposes interleaved with context h-chunk 0 matmuls
                pt = ptp.tile([P, LkB, P], BF16)
                cps0 = cq_ps.tile([P, 512], F32, tag="cq", name="cps0")
                for g4 in range(LkB // 4):
                    ps = t_ps.tile([P, 512], BF16, tag="tps", name="ps")
                    for j in range(4):
                        kb = g4 * 4 + j
                        nc.tensor.transpose(
                            ps[:, j * P : (j + 1) * P],
                            p16[:, kb * P : (kb + 1) * P],
                            id16,
                        )
                    nc.vector.tensor_copy(pt[:, g4 * 4 : (g4 + 1) * 4, :], ps)
                    for kb in range(g4 * 4, g4 * 4 + 4):
                        nc.tensor.matmul(
                            cps0,
                            pt[:, kb, :],
                            v_sb[:, kb, 0:512],
                            start=(kb == 0),
                            stop=(kb == LkB - 1),
                        )

                ctx_sb = ctxp.tile([P, H], F32)
                nc.vector.tensor_scalar_mul(ctx_sb[:, 0:512], cps0, recip)
                for h2 in range(1, HC2):
                    hsl = slice(h2 * 512, (h2 + 1) * 512)
                    cps = cq_ps.tile([P, 512], F32, tag="cq", name="cps")
                    for kb in range(LkB):
                        nc.tensor.matmul(
                            cps,
                            pt[:, kb, :],
                            v_sb[:, kb, hsl],
                            start=(kb == 0),
                            stop=(kb == LkB - 1),
                        )
                    nc.vector.tensor_scalar_mul(ctx_sb[:, hsl], cps, recip)
                nc.scalar.dma_start(out=ctx_d[qsl, :], in_=ctx_sb)

    _split_wide_sync_waits(nc)
    return nc


_nc_cache = {}


def _get_nc(Lq, Lk, H):
    key = (Lq, Lk, H)
    if key not in _nc_cache:
        _nc_cache[key] = build_attention_nc(Lq, Lk, H)
    return _nc_cache[key]


def kernel(query, key, value, mask, W, trace=False):
    query = np.ascontiguousarray(np.asarray(query, dtype=np.float32))
    key = np.ascontiguousarray(np.asarray(key, dtype=np.float32))
    value = np.ascontiguousarray(np.asarray(value, dtype=np.float32))
    mask = np.ascontiguousarray(np.asarray(mask, dtype=np.int32))
    W = np.ascontiguousarray(np.asarray(W, dtype=np.float32))

    B, Lq, H = query.shape
    Lk = key.shape[1]
    assert B == 8, f"expected B=8, got {B}"

    nc = _get_nc(Lq, Lk, H)
    in_maps = [
        {
            "query": query[b],
            "key": key[b],
            "value": value[b],
            "mask": mask[b],
            "W": W,
        }
        for b in range(B)
    ]
    res = run_bass_kernel_spmd(
        nc, in_maps, core_ids=list(range(B)), trace=trace
    )
    context = np.stack([r["context"] for r in res.results])
    attention = np.stack([r["attention"] for r in res.results])
    if trace:
        kernel.last_exec_time_ns = res.exec_time_ns
        kernel.last_results = res
    return context, attention


# revision 15
# speedup vs baseline: 1.2138x; 1.0035x over previous
"""GeneralAttention Trainium2 Bass kernel.

Computes, for each batch b (data-parallel, one batch per NeuronCore):
    key_t   = key @ W^T
    energy  = (query @ key_t^T) / sqrt(H)        [B, Lq, Lk]
    energy  = where(mask == 0, -1e10, energy)
    att     = softmax(energy, axis=-1)
    context = att @ value                        [B, Lq, H]
returns (context, attention).

Math used on-chip (identical up to fp rounding):
    energy  = (query @ (W/sqrt(H))) @ key^T      (transform Q, not K)
    p       = exp(energy + (mask-1)*1e10)        (masked entries -> exp(-1e10) == 0)
    att     = exp(energy + bias - ln(sum(p)))    (no row-max needed: |energy| < ~10)
    context = (p @ value) * (1/sum(p))
The additive mask is injected directly into the PSUM energy accumulation with
one extra matmul whose stationary operand is the 128x128 identity and whose
moving operand is the bias tile, so no vector-engine pass over [128, Lk] is
needed before the exponent.
"""

import math
import sys
from contextlib import ExitStack

for _p in ("/opt/trn_rl_repo",):
    if _p not in sys.path:
        sys.path.insert(0, _p)

import numpy as np

import concourse.bass as bass
import concourse.mybir as mybir
import concourse.tile as tile
from concourse.bass_utils import run_bass_kernel_spmd
from concourse.masks import make_identity

P = 128
NEG_SCALE = 1.0e10
F32 = mybir.dt.float32
BF16 = mybir.dt.bfloat16
I32 = mybir.dt.int32
Copy = mybir.ActivationFunctionType.Copy
Exp = mybir.ActivationFunctionType.Exp
Ln = mybir.ActivationFunctionType.Ln


_DMA_INSTS = (
    mybir.InstDMACopy,
    mybir.InstDMA,
    mybir.InstDmaTransposeAnt,
    mybir.InstDMAGatherAnt,
    mybir.InstDMAScatterAddAnt,
)


def _split_wide_sync_waits(nc, cap=1):
    """walrus in this container rejects >1 sync-wait command per instruction.
    Move excess waits onto preceding single-wait nops on the same engine —
    semantically identical (the sequencer stalls either way), just encoded
    across several instructions.  Excess sem updates on non-DMA instructions
    move to a following nop (same engine, in-order completion).  DMA updates
    are never moved (they fire at transfer completion, a nop would not)."""
    n_wait = n_upd = 0
    for f in nc.m.functions:
        for blk in f.blocks:
            new_insts = []
            for inst in blk.instructions:
                si = inst.sync_info
                if si is not None and si.on_wait and len(si.on_wait) > cap:
                    waits = list(si.on_wait)
                    for w in waits[:-cap]:
                        nop = mybir.InstNoOp(
                            name=f"waitsplit-{nc.next_id()}",
                            ins=[],
                            outs=[],
                            engine=inst.engine,
                            sync_info=mybir.SyncInfo(on_wait=[w], on_update=[]),
                        )
                        new_insts.append(nop)
                        n_wait += 1
                    si.on_wait = waits[-cap:]
                new_insts.append(inst)
                if (
                    si is not None
                    and si.on_update
                    and len(si.on_update) > cap
                    and not isinstance(inst, _DMA_INSTS)
                ):
                    upds = list(si.on_update)
                    si.on_update = upds[:cap]
                    for u in upds[cap:]:
                        nop = mybir.InstNoOp(
                            name=f"updsplit-{nc.next_id()}",
                            ins=[],
                            outs=[],
                            engine=inst.engine,
                            sync_info=mybir.SyncInfo(on_wait=[], on_update=[u]),
                        )
                        new_insts.append(nop)
                        n_upd += 1
            blk.instructions = new_insts
    return n_wait, n_upd


def build_attention_nc(Lq, Lk, H, mask_bias_on_act=True):
    assert Lq % 512 == 0 and Lk % 512 == 0 and H % 512 == 0
    HB, LqB, LkB = H // P, Lq // P, Lk // P
    KC, QC, HC2 = Lk // 512, Lq // 512, H // 512
    scale = 1.0 / math.sqrt(H)

    nc = bass.Bass(trn_type="TRN2")
    q_d = nc.dram_tensor("query", [Lq, H], F32, kind="ExternalInput")
    k_d = nc.dram_tensor("key", [Lk, H], F32, kind="ExternalInput")
    v_d = nc.dram_tensor("value", [Lk, H], F32, kind="ExternalInput")
    m_d = nc.dram_tensor("mask", [Lq, Lk], I32, kind="ExternalInput")
    w_d = nc.dram_tensor("W", [H, H], F32, kind="ExternalInput")
    ctx_d = nc.dram_tensor("context", [Lq, H], F32, kind="ExternalOutput")
    att_d = nc.dram_tensor("attention", [Lq, Lk], F32, kind="ExternalOutput")

    with tile.TileContext(nc) as tc, ExitStack() as ctx:
        persist = ctx.enter_context(tc.tile_pool(name="persist", bufs=1))
        # W natural layout (o-partition, o-chunk, h), pre-scaled by 1/sqrt(H)
        w_sb = persist.tile([P, HB, H], BF16)
        # value natural layout (k-partition, k-block, h)
        v_sb = persist.tile([P, LkB, H], BF16)
        # key^T (h-partition, h-chunk, k)
        kt_sb = persist.tile([P, HB, Lk], BF16)
        # (Q @ W/sqrt(H))^T (h-partition, h-chunk, q)
        qwt_sb = persist.tile([P, HB, Lq], BF16)
        id16 = persist.tile([P, P], BF16)
        make_identity(nc, id16)

        # block-0 mask+bias live in an always-allocated pool and are computed
        # at kernel start, so the first energy matmuls after prep don't wait
        # for the prep SBUF region to be released and re-filled.
        warm = ctx.enter_context(tc.tile_pool(name="warm", bufs=1))
        mk0 = warm.tile([P, Lk], I32)
        bias0 = warm.tile([P, Lk], BF16)
        nc.scalar.dma_start(out=mk0, in_=m_d[0:P, :])
        if mask_bias_on_act:
            nc.scalar.activation(bias0, mk0, Copy, bias=-NEG_SCALE, scale=NEG_SCALE)
        else:
            nc.vector.tensor_scalar(
                bias0, mk0, NEG_SCALE, -NEG_SCALE,
                mybir.AluOpType.mult, mybir.AluOpType.add,
            )

        # flat PSUM pools shared by prep and main phases (same tags reuse
        # the same banks with no cross-phase handoff serialization):
        # energy 2x2 banks, transposes 2 banks, QWT/context 2 banks = 8.
        e_ps = ctx.enter_context(tc.tile_pool(name="e_ps", bufs=2, space="PSUM"))
        t_ps = ctx.enter_context(tc.tile_pool(name="t_ps", bufs=2, space="PSUM"))
        cq_ps = ctx.enter_context(tc.tile_pool(name="cq_ps", bufs=2, space="PSUM"))

        # ---------------- prep: W, V, K^T, q^T -> QW^T ----------------
        with (
            tc.tile_pool(name="prep_loads", bufs=2) as loads,
            tc.tile_pool(name="prep_misc", bufs=2) as prep_misc,
        ):
            # DMA queues are FIFO per HWDGE engine, and emission order sets
            # the scheduler's tie-break priority — so load in critical-path
            # order: first K-group 0 and W and Q-group 0 (together they
            # unblock the first energy matmuls ~30us in), then the remaining
            # K and Q groups, then V (only needed for the first context
            # matmul much later).  Mask rides the scalar-engine queue.
            def k_group(g):
                kf = loads.tile([P, 4, H], F32, tag="ldf", name="kf")
                if g == 0:
                    # per-tile DMAs: transposes of tile j start as soon as
                    # its slice lands, shortening the cold-start ramp
                    for j in range(4):
                        nc.sync.dma_start(
                            out=kf[:, j, :],
                            in_=k_d[(g * 4 + j) * P : (g * 4 + j + 1) * P, :],
                        )
                else:
                    nc.sync.dma_start(
                        out=kf,
                        in_=k_d[g * 512 : (g + 1) * 512, :].rearrange(
                            "(j p) h -> p j h", p=P
                        ),
                    )
                kb16 = loads.tile([P, 4, H], BF16, tag="ld16", name="kb16")
                nc.vector.tensor_copy(kb16, kf)
                for hc in range(HB):
                    ps = t_ps.tile([P, 512], BF16, tag="tps", name="ps")
                    for j in range(4):
                        nc.tensor.transpose(
                            ps[:, j * P : (j + 1) * P],
                            kb16[:, j, hc * P : (hc + 1) * P],
                            id16,
                        )
                    nc.vector.tensor_copy(kt_sb[:, hc, g * 512 : (g + 1) * 512], ps)

            def q_group(g):
                qf = loads.tile([P, 4, H], F32, tag="ldf", name="qf")
                if g == 0:
                    for j in range(4):
                        nc.sync.dma_start(
                            out=qf[:, j, :],
                            in_=q_d[(g * 4 + j) * P : (g * 4 + j + 1) * P, :],
                        )
                else:
                    nc.sync.dma_start(
                        out=qf,
                        in_=q_d[g * 512 : (g + 1) * 512, :].rearrange(
                            "(j p) h -> p j h", p=P
                        ),
                    )
                qb16 = loads.tile([P, 4, H], BF16, tag="ld16", name="qb16")
                nc.scalar.copy(qb16, qf)
                qt = loads.tile([P, HB, 512], BF16, tag="qt", name="qt")
                for oc in range(HB):
                    ps = t_ps.tile([P, 512], BF16, tag="tps", name="ps")
                    for j in range(4):
                        nc.tensor.transpose(
                            ps[:, j * P : (j + 1) * P],
                            qb16[:, j, oc * P : (oc + 1) * P],
                            id16,
                        )
                    nc.scalar.copy(qt[:, oc, :], ps)
                for hc in range(HB):
                    qw = cq_ps.tile([P, 512], F32, tag="cq", name="qw")
                    for oc in range(HB):
                        nc.tensor.matmul(
                            qw,
                            w_sb[:, oc, hc * P : (hc + 1) * P],
                            qt[:, oc, :],
                            start=(oc == 0),
                            stop=(oc == HB - 1),
                        )
                    nc.scalar.copy(qwt_sb[:, hc, g * 512 : (g + 1) * 512], qw)

            k_group(0)
            for oc in range(HB):
                wf = prep_misc.tile([P, H], F32, tag="wf")
                nc.sync.dma_start(out=wf, in_=w_d[oc * P : (oc + 1) * P, :])
                nc.scalar.activation(w_sb[:, oc, :], wf, Copy, scale=scale)
            q_group(0)
            for g in range(1, LkB // 4):
                k_group(g)
                q_group(g)
            for g in range(LkB // 4, QC):
                q_group(g)

            for kb in range(LkB):
                vf = prep_misc.tile([P, H], F32, tag="vf")
                nc.sync.dma_start(out=vf, in_=v_d[kb * P : (kb + 1) * P, :])
                nc.vector.tensor_copy(v_sb[:, kb, :], vf)

        # ---------------- main loop over q row-blocks ----------------
        # The energy row [P, Lk] is computed in two PSUM halves (2 banks
        # each, bufs=2): exp of half 0 runs on ACT while half 1's matmuls
        # stream on PE, and the half-0 banks recycle for the next block
        # without waiting on the whole row's exp.  pT transposes interleave
        # with the first h-chunk of context matmuls so the PE never sits on
        # a PSUM-evacuation wait.
        EH = Lk // 2          # elements per energy half
        KC2 = max(1, KC // 2)  # 512-chunks per half
        with (
            tc.tile_pool(name="maskp", bufs=2) as maskp,
            tc.tile_pool(name="biasp", bufs=2) as biasp,
            tc.tile_pool(name="pp", bufs=2) as pp,
            tc.tile_pool(name="attp", bufs=2) as attp,
            tc.tile_pool(name="ptp", bufs=2) as ptp,
            tc.tile_pool(name="ctxp", bufs=2) as ctxp,
            tc.tile_pool(name="sums", bufs=3) as sums,
        ):
            for qb in range(LqB):
                qsl = slice(qb * P, (qb + 1) * P)
                if qb == 0:
                    bias = bias0
                else:
                    mk = maskp.tile([P, Lk], I32)
                    nc.scalar.dma_start(out=mk, in_=m_d[qsl, :])
                    bias = biasp.tile([P, Lk], BF16)
                    if mask_bias_on_act:
                        nc.scalar.activation(
                            bias, mk, Copy, bias=-NEG_SCALE, scale=NEG_SCALE
                        )
                    else:
                        nc.vector.tensor_scalar(
                            bias,
                            mk,
                            NEG_SCALE,
                            -NEG_SCALE,
                            mybir.AluOpType.mult,
                            mybir.AluOpType.add,
                        )

                p16 = pp.tile([P, Lk], BF16)
                halves = []
                for h in range(2 if KC > 1 else 1):
                    eng = e_ps.tile([P, EH if KC > 1 else Lk], F32, tag="eng")
                    for kc2 in range(KC2):
                        kc = h * KC2 + kc2
                        ksl = slice(kc * 512, (kc + 1) * 512)
                        esl = slice(kc2 * 512, (kc2 + 1) * 512)
                        for hc in range(HB):
                            nc.tensor.matmul(
                                eng[:, esl],
                                qwt_sb[:, hc, qsl],
                                kt_sb[:, hc, ksl],
                                start=(hc == 0),
                                stop=False,
                            )
                        nc.tensor.matmul(
                            eng[:, esl], id16, bias[:, ksl], start=False, stop=True
                        )
                    rs = sums.tile([P, 1], F32, tag=f"rsum{h}", name="rs")
                    nc.scalar.activation(
                        p16[:, h * EH : h * EH + eng.shape[-1]], eng, Exp,
                        accum_out=rs,
                    )
                    halves.append(rs)

                rsum = sums.tile([P, 1], F32, tag="rsum")
                if len(halves) == 2:
                    nc.vector.tensor_add(rsum, halves[0], halves[1])
                else:
                    nc.vector.tensor_copy(rsum, halves[0])
                recip = sums.tile([P, 1], F32, tag="recip")
                nc.vector.reciprocal(recip, rsum)

                att = attp.tile([P, Lk], F32)
                nc.vector.tensor_scalar_mul(att, p16, recip)
                nc.scalar.dma_start(out=att_d[qsl, :], in_=att)

                # pT transposes interleaved with context h-chunk 0 matmuls
                pt = ptp.tile([P, LkB, P], BF16)
                cps0 = cq_ps.tile([P, 512], F32, tag="cq", name="cps0")
                for g4 in range(LkB // 4):
                    ps = t_ps.tile([P, 512], BF16, tag="tps", name="ps")
                    for j in range(4):
                        kb = g4 * 4 + j
                        nc.tensor.transpose(
                            ps[:, j * P : (j + 1) * P],
                            p16[:, kb * P : (kb + 1) * P],
                            id16,
                        )
                    nc.vector.tensor_copy(pt[:, g4 * 4 : (g4 + 1) * 4, :], ps)
                    for kb in range(g4 * 4, g4 * 4 + 4):
                        nc.tensor.matmul(
                            cps0,
                            pt[:, kb, :],
                            v_sb[:, kb, 0:512],
                            start=(kb == 0),
                            stop=(kb == LkB - 1),
                        )

                ctx_sb = ctxp.tile([P, H], F32)
                nc.vector.tensor_scalar_mul(ctx_sb[:, 0:512], cps0, recip)
                for h2 in range(1, HC2):
                    hsl = slice(h2 * 512, (h2 + 1) * 512)
                    cps = cq_ps.tile([P, 512], F32, tag="cq", name="cps")
                    for kb in range(LkB):
                        nc.tensor.matmul(
                            cps,
                            pt[:, kb, :],
                            v_sb[:, kb, hsl],
                            start=(kb == 0),
                            stop=(kb == LkB - 1),
                        )
                    nc.vector.tensor_scalar_mul(ctx_sb[:, hsl], cps, recip)
                nc.scalar.dma_start(out=ctx_d[qsl, :], in_=ctx_sb)

    _split_wide_sync_waits(nc)
    return nc


_nc_cache = {}


def _get_nc(Lq, Lk, H):
    key = (Lq, Lk, H)
    if key not in _nc_cache:
        _nc_cache[key] = build_attention_nc(Lq, Lk, H)
    return _nc_cache[key]


def kernel(query, key, value, mask, W, trace=False):
    query = np.ascontiguousarray(np.asarray(query, dtype=np.float32))
    key = np.ascontiguousarray(np.asarray(key, dtype=np.float32))
    value = np.ascontiguousarray(np.asarray(value, dtype=np.float32))
    mask = np.ascontiguousarray(np.asarray(mask, dtype=np.int32))
    W = np.ascontiguousarray(np.asarray(W, dtype=np.float32))

    B, Lq, H = query.shape
    Lk = key.shape[1]
    assert B == 8, f"expected B=8, got {B}"

    nc = _get_nc(Lq, Lk, H)
    in_maps = [
        {
            "query": query[b],
            "key": key[b],
            "value": value[b],
            "mask": mask[b],
            "W": W,
        }
        for b in range(B)
    ]
    res = run_bass_kernel_spmd(
        nc, in_maps, core_ids=list(range(B)), trace=trace
    )
    context = np.stack([r["context"] for r in res.results])
    attention = np.stack([r["attention"] for r in res.results])
    if trace:
        kernel.last_exec_time_ns = res.exec_time_ns
        kernel.last_results = res
    return context, attention


# revision 16
# speedup vs baseline: 1.2462x; 1.0267x over previous
"""GeneralAttention Trainium2 Bass kernel.

Computes, for each batch b (data-parallel, one batch per NeuronCore):
    key_t   = key @ W^T
    energy  = (query @ key_t^T) / sqrt(H)        [B, Lq, Lk]
    energy  = where(mask == 0, -1e10, energy)
    att     = softmax(energy, axis=-1)
    context = att @ value                        [B, Lq, H]
returns (context, attention).

Math used on-chip (identical up to fp rounding):
    energy  = (query @ (W/sqrt(H))) @ key^T      (transform Q, not K)
    p       = exp(energy + (mask-1)*1e10)        (masked entries -> exp(-1e10) == 0)
    att     = exp(energy + bias - ln(sum(p)))    (no row-max needed: |energy| < ~10)
    context = (p @ value) * (1/sum(p))
The additive mask is injected directly into the PSUM energy accumulation with
one extra matmul whose stationary operand is the 128x128 identity and whose
moving operand is the bias tile, so no vector-engine pass over [128, Lk] is
needed before the exponent.
"""

import math
import sys
from contextlib import ExitStack

for _p in ("/opt/trn_rl_repo",):
    if _p not in sys.path:
        sys.path.insert(0, _p)

import numpy as np

import concourse.bass as bass
import concourse.mybir as mybir
import concourse.tile as tile
from concourse.bass_utils import run_bass_kernel_spmd
from concourse.masks import make_identity

P = 128
NEG_SCALE = 1.0e10
F32 = mybir.dt.float32
BF16 = mybir.dt.bfloat16
I32 = mybir.dt.int32
Copy = mybir.ActivationFunctionType.Copy
Exp = mybir.ActivationFunctionType.Exp
Ln = mybir.ActivationFunctionType.Ln


_DMA_INSTS = (
    mybir.InstDMACopy,
    mybir.InstDMA,
    mybir.InstDmaTransposeAnt,
    mybir.InstDMAGatherAnt,
    mybir.InstDMAScatterAddAnt,
)


def _split_wide_sync_waits(nc, cap=1):
    """walrus in this container rejects >1 sync-wait command per instruction.
    Move excess waits onto preceding single-wait nops on the same engine —
    semantically identical (the sequencer stalls either way), just encoded
    across several instructions.  Excess sem updates on non-DMA instructions
    move to a following nop (same engine, in-order completion).  DMA updates
    are never moved (they fire at transfer completion, a nop would not)."""
    n_wait = n_upd = 0
    for f in nc.m.functions:
        for blk in f.blocks:
            new_insts = []
            for inst in blk.instructions:
                si = inst.sync_info
                if si is not None and si.on_wait and len(si.on_wait) > cap:
                    waits = list(si.on_wait)
                    for w in waits[:-cap]:
                        nop = mybir.InstNoOp(
                            name=f"waitsplit-{nc.next_id()}",
                            ins=[],
                            outs=[],
                            engine=inst.engine,
                            sync_info=mybir.SyncInfo(on_wait=[w], on_update=[]),
                        )
                        new_insts.append(nop)
                        n_wait += 1
                    si.on_wait = waits[-cap:]
                new_insts.append(inst)
                if (
                    si is not None
                    and si.on_update
                    and len(si.on_update) > cap
                    and not isinstance(inst, _DMA_INSTS)
                ):
                    upds = list(si.on_update)
                    si.on_update = upds[:cap]
                    for u in upds[cap:]:
                        nop = mybir.InstNoOp(
                            name=f"updsplit-{nc.next_id()}",
                            ins=[],
                            outs=[],
                            engine=inst.engine,
                            sync_info=mybir.SyncInfo(on_wait=[], on_update=[u]),
                        )
                        new_insts.append(nop)
                        n_upd += 1
            blk.instructions = new_insts
    return n_wait, n_upd


def build_attention_nc(Lq, Lk, H, mask_bias_on_act=True):
    assert Lq % 512 == 0 and Lk % 512 == 0 and H % 512 == 0
    HB, LqB, LkB = H // P, Lq // P, Lk // P
    KC, QC, HC2 = Lk // 512, Lq // 512, H // 512
    scale = 1.0 / math.sqrt(H)

    nc = bass.Bass(trn_type="TRN2")
    q_d = nc.dram_tensor("query", [Lq, H], F32, kind="ExternalInput")
    k_d = nc.dram_tensor("key", [Lk, H], F32, kind="ExternalInput")
    v_d = nc.dram_tensor("value", [Lk, H], F32, kind="ExternalInput")
    m_d = nc.dram_tensor("mask", [Lq, Lk], I32, kind="ExternalInput")
    w_d = nc.dram_tensor("W", [H, H], F32, kind="ExternalInput")
    ctx_d = nc.dram_tensor("context", [Lq, H], F32, kind="ExternalOutput")
    att_d = nc.dram_tensor("attention", [Lq, Lk], F32, kind="ExternalOutput")

    with tile.TileContext(nc) as tc, ExitStack() as ctx:
        persist = ctx.enter_context(tc.tile_pool(name="persist", bufs=1))
        # W natural layout (o-partition, o-chunk, h), pre-scaled by 1/sqrt(H)
        w_sb = persist.tile([P, HB, H], BF16)
        # value natural layout (k-partition, k-block, h)
        v_sb = persist.tile([P, LkB, H], BF16)
        # key^T (h-partition, h-chunk, k)
        kt_sb = persist.tile([P, HB, Lk], BF16)
        # (Q @ W/sqrt(H))^T (h-partition, h-chunk, q)
        qwt_sb = persist.tile([P, HB, Lq], BF16)
        id16 = persist.tile([P, P], BF16)
        make_identity(nc, id16)

        # block-0 mask+bias live in an always-allocated pool and are computed
        # at kernel start, so the first energy matmuls after prep don't wait
        # for the prep SBUF region to be released and re-filled.
        warm = ctx.enter_context(tc.tile_pool(name="warm", bufs=1))
        mk0 = warm.tile([P, Lk], I32)
        bias0 = warm.tile([P, Lk], BF16)

        def emit_warm_bias():
            nc.scalar.dma_start(out=mk0, in_=m_d[0:P, :])
            if mask_bias_on_act:
                nc.scalar.activation(
                    bias0, mk0, Copy, bias=-NEG_SCALE, scale=NEG_SCALE
                )
            else:
                nc.vector.tensor_scalar(
                    bias0, mk0, NEG_SCALE, -NEG_SCALE,
                    mybir.AluOpType.mult, mybir.AluOpType.add,
                )

        # flat PSUM pools shared by prep and main phases (same tags reuse
        # the same banks with no cross-phase handoff serialization):
        # energy 2x2 banks, transposes 2 banks, QWT/context 2 banks = 8.
        e_ps = ctx.enter_context(tc.tile_pool(name="e_ps", bufs=2, space="PSUM"))
        t_ps = ctx.enter_context(tc.tile_pool(name="t_ps", bufs=2, space="PSUM"))
        cq_ps = ctx.enter_context(tc.tile_pool(name="cq_ps", bufs=2, space="PSUM"))

        # ---------------- prep: W, V, K^T, q^T -> QW^T ----------------
        with (
            tc.tile_pool(name="prep_loads", bufs=2) as loads,
            tc.tile_pool(name="prep_misc", bufs=2) as prep_misc,
        ):
            # DMA queues are FIFO per HWDGE engine, and emission order sets
            # the scheduler's tie-break priority — so load in critical-path
            # order: first K-group 0 and W and Q-group 0 (together they
            # unblock the first energy matmuls ~30us in), then the remaining
            # K and Q groups, then V (only needed for the first context
            # matmul much later).  Mask rides the scalar-engine queue.
            def k_group(g):
                kf = loads.tile([P, 4, H], F32, tag="ldf", name="kf")
                if g == 0:
                    # per-tile DMAs: transposes of tile j start as soon as
                    # its slice lands, shortening the cold-start ramp
                    for j in range(4):
                        nc.sync.dma_start(
                            out=kf[:, j, :],
                            in_=k_d[(g * 4 + j) * P : (g * 4 + j + 1) * P, :],
                        )
                else:
                    nc.sync.dma_start(
                        out=kf,
                        in_=k_d[g * 512 : (g + 1) * 512, :].rearrange(
                            "(j p) h -> p j h", p=P
                        ),
                    )
                kb16 = loads.tile([P, 4, H], BF16, tag="ld16", name="kb16")
                nc.vector.tensor_copy(kb16, kf)
                for hc in range(HB):
                    ps = t_ps.tile([P, 512], BF16, tag="tps", name="ps")
                    for j in range(4):
                        nc.tensor.transpose(
                            ps[:, j * P : (j + 1) * P],
                            kb16[:, j, hc * P : (hc + 1) * P],
                            id16,
                        )
                    nc.vector.tensor_copy(kt_sb[:, hc, g * 512 : (g + 1) * 512], ps)

            def q_group(g):
                qf = loads.tile([P, 4, H], F32, tag="ldf", name="qf")
                if g == 0:
                    for j in range(4):
                        nc.sync.dma_start(
                            out=qf[:, j, :],
                            in_=q_d[(g * 4 + j) * P : (g * 4 + j + 1) * P, :],
                        )
                else:
                    nc.sync.dma_start(
                        out=qf,
                        in_=q_d[g * 512 : (g + 1) * 512, :].rearrange(
                            "(j p) h -> p j h", p=P
                        ),
                    )
                qb16 = loads.tile([P, 4, H], BF16, tag="ld16", name="qb16")
                nc.scalar.copy(qb16, qf)
                qt = loads.tile([P, HB, 512], BF16, tag="qt", name="qt")
                for oc in range(HB):
                    ps = t_ps.tile([P, 512], BF16, tag="tps", name="ps")
                    for j in range(4):
                        nc.tensor.transpose(
                            ps[:, j * P : (j + 1) * P],
                            qb16[:, j, oc * P : (oc + 1) * P],
                            id16,
                        )
                    nc.scalar.copy(qt[:, oc, :], ps)
                for hc in range(HB):
                    qw = cq_ps.tile([P, 512], F32, tag="cq", name="qw")
                    for oc in range(HB):
                        nc.tensor.matmul(
                            qw,
                            w_sb[:, oc, hc * P : (hc + 1) * P],
                            qt[:, oc, :],
                            start=(oc == 0),
                            stop=(oc == HB - 1),
                        )
                    nc.scalar.copy(qwt_sb[:, hc, g * 512 : (g + 1) * 512], qw)

            k_group(0)
            for oc in range(HB):
                wf = prep_misc.tile([P, H], F32, tag="wf")
                nc.sync.dma_start(out=wf, in_=w_d[oc * P : (oc + 1) * P, :])
                nc.scalar.activation(w_sb[:, oc, :], wf, Copy, scale=scale)
            q_group(0)
            emit_warm_bias()
            for g in range(1, LkB // 4):
                k_group(g)
                q_group(g)
            for g in range(LkB // 4, QC):
                q_group(g)

            for kb in range(LkB):
                vf = prep_misc.tile([P, H], F32, tag="vf")
                nc.sync.dma_start(out=vf, in_=v_d[kb * P : (kb + 1) * P, :])
                nc.vector.tensor_copy(v_sb[:, kb, :], vf)

        # ---------------- main loop over q row-blocks ----------------
        # The energy row [P, Lk] is computed in two PSUM halves (2 banks
        # each, bufs=2): exp of half 0 runs on ACT while half 1's matmuls
        # stream on PE, and the half-0 banks recycle for the next block
        # without waiting on the whole row's exp.  pT transposes interleave
        # with the first h-chunk of context matmuls so the PE never sits on
        # a PSUM-evacuation wait.
        EH = Lk // 2          # elements per energy half
        KC2 = max(1, KC // 2)  # 512-chunks per half
        with (
            tc.tile_pool(name="maskp", bufs=2) as maskp,
            tc.tile_pool(name="biasp", bufs=2) as biasp,
            tc.tile_pool(name="pp", bufs=2) as pp,
            tc.tile_pool(name="attp", bufs=2) as attp,
            tc.tile_pool(name="ptp", bufs=2) as ptp,
            tc.tile_pool(name="ctxp", bufs=2) as ctxp,
            tc.tile_pool(name="sums", bufs=3) as sums,
        ):
            for qb in range(LqB):
                qsl = slice(qb * P, (qb + 1) * P)
                if qb == 0:
                    bias = bias0
                else:
                    mk = maskp.tile([P, Lk], I32)
                    nc.scalar.dma_start(out=mk, in_=m_d[qsl, :])
                    bias = biasp.tile([P, Lk], BF16)
                    if mask_bias_on_act:
                        nc.scalar.activation(
                            bias, mk, Copy, bias=-NEG_SCALE, scale=NEG_SCALE
                        )
                    else:
                        nc.vector.tensor_scalar(
                            bias,
                            mk,
                            NEG_SCALE,
                            -NEG_SCALE,
                            mybir.AluOpType.mult,
                            mybir.AluOpType.add,
                        )

                p16 = pp.tile([P, Lk], BF16)
                halves = []
                for h in range(2 if KC > 1 else 1):
                    eng = e_ps.tile([P, EH if KC > 1 else Lk], F32, tag="eng")
                    for kc2 in range(KC2):
                        kc = h * KC2 + kc2
                        ksl = slice(kc * 512, (kc + 1) * 512)
                        esl = slice(kc2 * 512, (kc2 + 1) * 512)
                        for hc in range(HB):
                            nc.tensor.matmul(
                                eng[:, esl],
                                qwt_sb[:, hc, qsl],
                                kt_sb[:, hc, ksl],
                                start=(hc == 0),
                                stop=False,
                            )
                        nc.tensor.matmul(
                            eng[:, esl], id16, bias[:, ksl], start=False, stop=True
                        )
                    rs = sums.tile([P, 1], F32, tag=f"rsum{h}", name="rs")
                    nc.scalar.activation(
                        p16[:, h * EH : h * EH + eng.shape[-1]], eng, Exp,
                        accum_out=rs,
                    )
                    halves.append(rs)

                rsum = sums.tile([P, 1], F32, tag="rsum")
                if len(halves) == 2:
                    nc.vector.tensor_add(rsum, halves[0], halves[1])
                else:
                    nc.vector.tensor_copy(rsum, halves[0])
                recip = sums.tile([P, 1], F32, tag="recip")
                nc.vector.reciprocal(recip, rsum)

                att = attp.tile([P, Lk], F32)
                nc.vector.tensor_scalar_mul(att, p16, recip)
                nc.scalar.dma_start(out=att_d[qsl, :], in_=att)

                # pT transposes interleaved with context h-chunk 0 matmuls
                pt = ptp.tile([P, LkB, P], BF16)
                cps0 = cq_ps.tile([P, 512], F32, tag="cq", name="cps0")
                for g4 in range(LkB // 4):
                    ps = t_ps.tile([P, 512], BF16, tag="tps", name="ps")
                    for j in range(4):
                        kb = g4 * 4 + j
                        nc.tensor.transpose(
                            ps[:, j * P : (j + 1) * P],
                            p16[:, kb * P : (kb + 1) * P],
                            id16,
                        )
                    nc.vector.tensor_copy(pt[:, g4 * 4 : (g4 + 1) * 4, :], ps)
                    for kb in range(g4 * 4, g4 * 4 + 4):
                        nc.tensor.matmul(
                            cps0,
                            pt[:, kb, :],
                            v_sb[:, kb, 0:512],
                            start=(kb == 0),
                            stop=(kb == LkB - 1),
                        )

                ctx_sb = ctxp.tile([P, H], F32)
                nc.vector.tensor_scalar_mul(ctx_sb[:, 0:512], cps0, recip)
                for h2 in range(1, HC2):
                    hsl = slice(h2 * 512, (h2 + 1) * 512)
                    cps = cq_ps.tile([P, 512], F32, tag="cq", name="cps")
                    for kb in range(LkB):
                        nc.tensor.matmul(
                            cps,
                            pt[:, kb, :],
                            v_sb[:, kb, hsl],
                            start=(kb == 0),
                            stop=(kb == LkB - 1),
                        )
                    nc.vector.tensor_scalar_mul(ctx_sb[:, hsl], cps, recip)
                nc.scalar.dma_start(out=ctx_d[qsl, :], in_=ctx_sb)

    _split_wide_sync_waits(nc)
    return nc


_nc_cache = {}


def _get_nc(Lq, Lk, H):
    key = (Lq, Lk, H)
    if key not in _nc_cache:
        _nc_cache[key] = build_attention_nc(Lq, Lk, H)
    return _nc_cache[key]


def kernel(query, key, value, mask, W, trace=False):
    query = np.ascontiguousarray(np.asarray(query, dtype=np.float32))
    key = np.ascontiguousarray(np.asarray(key, dtype=np.float32))
    value = np.ascontiguousarray(np.asarray(value, dtype=np.float32))
    mask = np.ascontiguousarray(np.asarray(mask, dtype=np.int32))
    W = np.ascontiguousarray(np.asarray(W, dtype=np.float32))

    B, Lq, H = query.shape
    Lk = key.shape[1]
    assert B == 8, f"expected B=8, got {B}"

    nc = _get_nc(Lq, Lk, H)
    in_maps = [
        {
            "query": query[b],
            "key": key[b],
            "value": value[b],
            "mask": mask[b],
            "W": W,
        }
        for b in range(B)
    ]
    res = run_bass_kernel_spmd(
        nc, in_maps, core_ids=list(range(B)), trace=trace
    )
    context = np.stack([r["context"] for r in res.results])
    attention = np.stack([r["attention"] for r in res.results])
    if trace:
        kernel.last_exec_time_ns = res.exec_time_ns
        kernel.last_results = res
    return context, attention


# revision 17
# speedup vs baseline: 1.2672x; 1.0169x over previous
"""GeneralAttention Trainium2 Bass kernel.

Computes, for each batch b (data-parallel, one batch per NeuronCore):
    key_t   = key @ W^T
    energy  = (query @ key_t^T) / sqrt(H)        [B, Lq, Lk]
    energy  = where(mask == 0, -1e10, energy)
    att     = softmax(energy, axis=-1)
    context = att @ value                        [B, Lq, H]
returns (context, attention).

Math used on-chip (identical up to fp rounding):
    energy  = (query @ (W/sqrt(H))) @ key^T      (transform Q, not K)
    p       = exp(energy + (mask-1)*1e10)        (masked entries -> exp(-1e10) == 0)
    att     = exp(energy + bias - ln(sum(p)))    (no row-max needed: |energy| < ~10)
    context = (p @ value) * (1/sum(p))
The additive mask is injected directly into the PSUM energy accumulation with
one extra matmul whose stationary operand is the 128x128 identity and whose
moving operand is the bias tile, so no vector-engine pass over [128, Lk] is
needed before the exponent.
"""

import math
import sys
from contextlib import ExitStack

for _p in ("/opt/trn_rl_repo",):
    if _p not in sys.path:
        sys.path.insert(0, _p)

import numpy as np

import concourse.bass as bass
import concourse.mybir as mybir
import concourse.tile as tile
from concourse.bass_utils import run_bass_kernel_spmd
from concourse.masks import make_identity

P = 128
NEG_SCALE = 1.0e10
F32 = mybir.dt.float32
BF16 = mybir.dt.bfloat16
I32 = mybir.dt.int32
Copy = mybir.ActivationFunctionType.Copy
Exp = mybir.ActivationFunctionType.Exp
Ln = mybir.ActivationFunctionType.Ln


_DMA_INSTS = (
    mybir.InstDMACopy,
    mybir.InstDMA,
    mybir.InstDmaTransposeAnt,
    mybir.InstDMAGatherAnt,
    mybir.InstDMAScatterAddAnt,
)


def _split_wide_sync_waits(nc, cap=1):
    """walrus in this container rejects >1 sync-wait command per instruction.
    Move excess waits onto preceding single-wait nops on the same engine —
    semantically identical (the sequencer stalls either way), just encoded
    across several instructions.  Excess sem updates on non-DMA instructions
    move to a following nop (same engine, in-order completion).  DMA updates
    are never moved (they fire at transfer completion, a nop would not)."""
    n_wait = n_upd = 0
    for f in nc.m.functions:
        for blk in f.blocks:
            new_insts = []
            for inst in blk.instructions:
                si = inst.sync_info
                if si is not None and si.on_wait and len(si.on_wait) > cap:
                    waits = list(si.on_wait)
                    for w in waits[:-cap]:
                        nop = mybir.InstNoOp(
                            name=f"waitsplit-{nc.next_id()}",
                            ins=[],
                            outs=[],
                            engine=inst.engine,
                            sync_info=mybir.SyncInfo(on_wait=[w], on_update=[]),
                        )
                        new_insts.append(nop)
                        n_wait += 1
                    si.on_wait = waits[-cap:]
                new_insts.append(inst)
                if (
                    si is not None
                    and si.on_update
                    and len(si.on_update) > cap
                    and not isinstance(inst, _DMA_INSTS)
                ):
                    upds = list(si.on_update)
                    si.on_update = upds[:cap]
                    for u in upds[cap:]:
                        nop = mybir.InstNoOp(
                            name=f"updsplit-{nc.next_id()}",
                            ins=[],
                            outs=[],
                            engine=inst.engine,
                            sync_info=mybir.SyncInfo(on_wait=[], on_update=[u]),
                        )
                        new_insts.append(nop)
                        n_upd += 1
            blk.instructions = new_insts
    return n_wait, n_upd


def build_attention_nc(Lq, Lk, H, mask_bias_on_act=True):
    assert Lq % 512 == 0 and Lk % 512 == 0 and H % 512 == 0
    HB, LqB, LkB = H // P, Lq // P, Lk // P
    KC, QC, HC2 = Lk // 512, Lq // 512, H // 512
    scale = 1.0 / math.sqrt(H)

    nc = bass.Bass(trn_type="TRN2")
    q_d = nc.dram_tensor("query", [Lq, H], F32, kind="ExternalInput")
    k_d = nc.dram_tensor("key", [Lk, H], F32, kind="ExternalInput")
    v_d = nc.dram_tensor("value", [Lk, H], F32, kind="ExternalInput")
    m_d = nc.dram_tensor("mask", [Lq, Lk], I32, kind="ExternalInput")
    w_d = nc.dram_tensor("W", [H, H], F32, kind="ExternalInput")
    ctx_d = nc.dram_tensor("context", [Lq, H], F32, kind="ExternalOutput")
    att_d = nc.dram_tensor("attention", [Lq, Lk], F32, kind="ExternalOutput")

    with tile.TileContext(nc) as tc, ExitStack() as ctx:
        persist = ctx.enter_context(tc.tile_pool(name="persist", bufs=1))
        # W natural layout (o-partition, o-chunk, h), pre-scaled by 1/sqrt(H)
        w_sb = persist.tile([P, HB, H], BF16)
        # value natural layout (k-partition, k-block, h)
        v_sb = persist.tile([P, LkB, H], BF16)
        # key^T (h-partition, h-chunk, k)
        kt_sb = persist.tile([P, HB, Lk], BF16)
        # (Q @ W/sqrt(H))^T (h-partition, h-chunk, q)
        qwt_sb = persist.tile([P, HB, Lq], BF16)
        id16 = persist.tile([P, P], BF16)
        make_identity(nc, id16)

        # block-0 mask+bias live in an always-allocated pool and are computed
        # at kernel start, so the first energy matmuls after prep don't wait
        # for the prep SBUF region to be released and re-filled.
        warm = ctx.enter_context(tc.tile_pool(name="warm", bufs=1))
        mk0 = warm.tile([P, Lk], I32)
        bias0 = warm.tile([P, Lk], BF16)

        def emit_warm_bias():
            nc.scalar.dma_start(out=mk0, in_=m_d[0:P, :])
            if mask_bias_on_act:
                nc.scalar.activation(
                    bias0, mk0, Copy, bias=-NEG_SCALE, scale=NEG_SCALE
                )
            else:
                nc.vector.tensor_scalar(
                    bias0, mk0, NEG_SCALE, -NEG_SCALE,
                    mybir.AluOpType.mult, mybir.AluOpType.add,
                )

        # flat PSUM pools shared by prep and main phases (same tags reuse
        # the same banks with no cross-phase handoff serialization):
        # energy 2x2 banks, transposes 2 banks, QWT/context 2 banks = 8.
        e_ps = ctx.enter_context(tc.tile_pool(name="e_ps", bufs=2, space="PSUM"))
        t_ps = ctx.enter_context(tc.tile_pool(name="t_ps", bufs=2, space="PSUM"))
        cq_ps = ctx.enter_context(tc.tile_pool(name="cq_ps", bufs=2, space="PSUM"))

        # ---------------- prep: W, V, K^T, q^T -> QW^T ----------------
        with tc.tile_pool(name="prep_loads", bufs=2) as loads:
            # DMA queues are FIFO per HWDGE engine, and emission order sets
            # the scheduler's tie-break priority — so load in critical-path
            # order: first K-group 0 and W and Q-group 0 (together they
            # unblock the first energy matmuls ~30us in), then the remaining
            # K and Q groups, then V (only needed for the first context
            # matmul much later).  Mask rides the scalar-engine queue.
            def k_group(g):
                kf = loads.tile([P, 4, H], F32, tag="ldf", name="kf")
                nc.sync.dma_start(
                    out=kf,
                    in_=k_d[g * 512 : (g + 1) * 512, :].rearrange(
                        "(j p) h -> p j h", p=P
                    ),
                )
                kb16 = loads.tile([P, 4, H], BF16, tag="ld16", name="kb16")
                nc.vector.tensor_copy(kb16, kf)
                for hc in range(HB):
                    ps = t_ps.tile([P, 512], BF16, tag="tps", name="ps")
                    for j in range(4):
                        nc.tensor.transpose(
                            ps[:, j * P : (j + 1) * P],
                            kb16[:, j, hc * P : (hc + 1) * P],
                            id16,
                        )
                    nc.vector.tensor_copy(kt_sb[:, hc, g * 512 : (g + 1) * 512], ps)

            def q_group(g):
                qf = loads.tile([P, 4, H], F32, tag="ldf", name="qf")
                nc.sync.dma_start(
                    out=qf,
                    in_=q_d[g * 512 : (g + 1) * 512, :].rearrange(
                        "(j p) h -> p j h", p=P
                    ),
                )
                qb16 = loads.tile([P, 4, H], BF16, tag="ld16", name="qb16")
                nc.vector.tensor_copy(qb16, qf)
                qt = loads.tile([P, HB, 512], BF16, tag="qt", name="qt")
                for oc in range(HB):
                    ps = t_ps.tile([P, 512], BF16, tag="tps", name="ps")
                    for j in range(4):
                        nc.tensor.transpose(
                            ps[:, j * P : (j + 1) * P],
                            qb16[:, j, oc * P : (oc + 1) * P],
                            id16,
                        )
                    nc.scalar.copy(qt[:, oc, :], ps)
                for hc in range(HB):
                    qw = cq_ps.tile([P, 512], F32, tag="cq", name="qw")
                    for oc in range(HB):
                        nc.tensor.matmul(
                            qw,
                            w_sb[:, oc, hc * P : (hc + 1) * P],
                            qt[:, oc, :],
                            start=(oc == 0),
                            stop=(oc == HB - 1),
                        )
                    nc.scalar.copy(qwt_sb[:, hc, g * 512 : (g + 1) * 512], qw)

            def w_group(g):
                wf = loads.tile([P, 4, H], F32, tag="ldf", name="wf")
                nc.scalar.dma_start(
                    out=wf,
                    in_=w_d[g * 512 : (g + 1) * 512, :].rearrange(
                        "(j p) h -> p j h", p=P
                    ),
                )
                for j in range(4):
                    nc.scalar.activation(
                        w_sb[:, g * 4 + j, :], wf[:, j, :], Copy, scale=scale
                    )

            def v_group(g):
                vf = loads.tile([P, 4, H], F32, tag="ldf", name="vf")
                nc.sync.dma_start(
                    out=vf,
                    in_=v_d[g * 512 : (g + 1) * 512, :].rearrange(
                        "(j p) h -> p j h", p=P
                    ),
                )
                nc.vector.tensor_copy(v_sb[:, g * 4 : (g + 1) * 4, :], vf)

            k_group(0)
            for g in range(HB // 4):
                w_group(g)
            q_group(0)
            emit_warm_bias()
            for g in range(1, LkB // 4):
                k_group(g)
                q_group(g)
            for g in range(LkB // 4, QC):
                q_group(g)
            for g in range(LkB // 4):
                v_group(g)

        # ---------------- main loop over q row-blocks ----------------
        # The energy row [P, Lk] is computed in two PSUM halves (2 banks
        # each, bufs=2): exp of half 0 runs on ACT while half 1's matmuls
        # stream on PE, and the half-0 banks recycle for the next block
        # without waiting on the whole row's exp.  pT transposes interleave
        # with the first h-chunk of context matmuls so the PE never sits on
        # a PSUM-evacuation wait.
        EH = Lk // 2          # elements per energy half
        KC2 = max(1, KC // 2)  # 512-chunks per half
        with (
            tc.tile_pool(name="maskp", bufs=2) as maskp,
            tc.tile_pool(name="biasp", bufs=2) as biasp,
            tc.tile_pool(name="pp", bufs=2) as pp,
            tc.tile_pool(name="attp", bufs=2) as attp,
            tc.tile_pool(name="ptp", bufs=2) as ptp,
            tc.tile_pool(name="ctxp", bufs=2) as ctxp,
            tc.tile_pool(name="sums", bufs=3) as sums,
        ):
            for qb in range(LqB):
                qsl = slice(qb * P, (qb + 1) * P)
                if qb == 0:
                    bias = bias0
                else:
                    mk = maskp.tile([P, Lk], I32)
                    nc.scalar.dma_start(out=mk, in_=m_d[qsl, :])
                    bias = biasp.tile([P, Lk], BF16)
                    if mask_bias_on_act:
                        nc.scalar.activation(
                            bias, mk, Copy, bias=-NEG_SCALE, scale=NEG_SCALE
                        )
                    else:
                        nc.vector.tensor_scalar(
                            bias,
                            mk,
                            NEG_SCALE,
                            -NEG_SCALE,
                            mybir.AluOpType.mult,
                            mybir.AluOpType.add,
                        )

                p16 = pp.tile([P, Lk], BF16)
                halves = []
                for h in range(2 if KC > 1 else 1):
                    eng = e_ps.tile([P, EH if KC > 1 else Lk], F32, tag="eng")
                    for kc2 in range(KC2):
                        kc = h * KC2 + kc2
                        ksl = slice(kc * 512, (kc + 1) * 512)
                        esl = slice(kc2 * 512, (kc2 + 1) * 512)
                        for hc in range(HB):
                            nc.tensor.matmul(
                                eng[:, esl],
                                qwt_sb[:, hc, qsl],
                                kt_sb[:, hc, ksl],
                                start=(hc == 0),
                                stop=False,
                            )
                        nc.tensor.matmul(
                            eng[:, esl], id16, bias[:, ksl], start=False, stop=True
                        )
                    rs = sums.tile([P, 1], F32, tag=f"rsum{h}", name="rs")
                    nc.scalar.activation(
                        p16[:, h * EH : h * EH + eng.shape[-1]], eng, Exp,
                        accum_out=rs,
                    )
                    halves.append(rs)

                rsum = sums.tile([P, 1], F32, tag="rsum")
                if len(halves) == 2:
                    nc.vector.tensor_add(rsum, halves[0], halves[1])
                else:
                    nc.vector.tensor_copy(rsum, halves[0])
                recip = sums.tile([P, 1], F32, tag="recip")
                nc.vector.reciprocal(recip, rsum)

                att = attp.tile([P, Lk], F32)
                nc.vector.tensor_scalar_mul(att, p16, recip)
                nc.scalar.dma_start(out=att_d[qsl, :], in_=att)

                # pT transposes interleaved with context h-chunk 0 matmuls
                pt = ptp.tile([P, LkB, P], BF16)
                cps0 = cq_ps.tile([P, 512], F32, tag="cq", name="cps0")
                for g4 in range(LkB // 4):
                    ps = t_ps.tile([P, 512], BF16, tag="tps", name="ps")
                    for j in range(4):
                        kb = g4 * 4 + j
                        nc.tensor.transpose(
                            ps[:, j * P : (j + 1) * P],
                            p16[:, kb * P : (kb + 1) * P],
                            id16,
                        )
                    nc.vector.tensor_copy(pt[:, g4 * 4 : (g4 + 1) * 4, :], ps)
                    for kb in range(g4 * 4, g4 * 4 + 4):
                        nc.tensor.matmul(
                            cps0,
                            pt[:, kb, :],
                            v_sb[:, kb, 0:512],
                            start=(kb == 0),
                            stop=(kb == LkB - 1),
                        )

                ctx_sb = ctxp.tile([P, H], F32)
                nc.vector.tensor_scalar_mul(ctx_sb[:, 0:512], cps0, recip)
                for h2 in range(1, HC2):
                    hsl = slice(h2 * 512, (h2 + 1) * 512)
                    cps = cq_ps.tile([P, 512], F32, tag="cq", name="cps")
                    for kb in range(LkB):
                        nc.tensor.matmul(
                            cps,
                            pt[:, kb, :],
                            v_sb[:, kb, hsl],
                            start=(kb == 0),
                            stop=(kb == LkB - 1),
                        )
                    nc.vector.tensor_scalar_mul(ctx_sb[:, hsl], cps, recip)
                nc.scalar.dma_start(out=ctx_d[qsl, :], in_=ctx_sb)

    _split_wide_sync_waits(nc)
    return nc


_nc_cache = {}


def _get_nc(Lq, Lk, H):
    key = (Lq, Lk, H)
    if key not in _nc_cache:
        _nc_cache[key] = build_attention_nc(Lq, Lk, H)
    return _nc_cache[key]


def kernel(query, key, value, mask, W, trace=False):
    query = np.ascontiguousarray(np.asarray(query, dtype=np.float32))
    key = np.ascontiguousarray(np.asarray(key, dtype=np.float32))
    value = np.ascontiguousarray(np.asarray(value, dtype=np.float32))
    mask = np.ascontiguousarray(np.asarray(mask, dtype=np.int32))
    W = np.ascontiguousarray(np.asarray(W, dtype=np.float32))

    B, Lq, H = query.shape
    Lk = key.shape[1]
    assert B == 8, f"expected B=8, got {B}"

    nc = _get_nc(Lq, Lk, H)
    in_maps = [
        {
            "query": query[b],
            "key": key[b],
            "value": value[b],
            "mask": mask[b],
            "W": W,
        }
        for b in range(B)
    ]
    res = run_bass_kernel_spmd(
        nc, in_maps, core_ids=list(range(B)), trace=trace
    )
    context = np.stack([r["context"] for r in res.results])
    attention = np.stack([r["attention"] for r in res.results])
    if trace:
        kernel.last_exec_time_ns = res.exec_time_ns
        kernel.last_results = res
    return context, attention
